# revision 27
# baseline (speedup 1.0000x reference)
"""Trainium2 Bass kernel for nn_Block_49220325212407 (dense transformer block).

Strategy (8 NeuronCores, SPMD single NEFF):
- Phases 1/2 (self-attn over T, cross-attn): data-parallel over the 32
  (b, n) frames -> 4 frames per core.
- One 8-way AllToAll (split in two halves for overlap) reshards to
  (b, t-chunk) ownership: core j owns t-rows [64j, 64j+64) of all frames.
- Phase 3 (frame attention over N=16): block-diagonal batched attention,
  8 t-rows per 128-col group, multiplicative 0/1 mask after exp.
- Phase 4 (MLP): per-token, computed in the resharded layout.

All activations live feature-major ([C, tokens] on SBUF partitions), so no
transposes are ever needed: LN stats via ones-column matmuls, broadcasts via
GpSimd partition_broadcast, softmax denominators via a ones-column appended
to V (v_aug), and the av matmul produces feature-major output directly.
Matmuls run in bf16 with f32 PSUM accumulation; the residual stream stays
f32 through phase 1 and is rounded to bf16 at the AllToAll and after
phase 3 (rel err ~6e-3 vs the f32 reference).
"""
import os
from contextlib import ExitStack

import numpy as np
import ml_dtypes

import concourse.bass as bass
import concourse.bass_isa as bass_isa
import concourse.tile as tile
from concourse import bacc, mybir
from concourse.bass_utils import run_bass_kernel_spmd

f32 = np.float32
BF = ml_dtypes.bfloat16
F32D = mybir.dt.float32
BF16D = mybir.dt.bfloat16
AF = mybir.ActivationFunctionType

P = 128
C = 768
CH = 6          # C / P
T = 512
H = 12
D = 64
NF = 4          # frames per core
NCORES = 8
TLOC = T // NCORES   # 64
HID = 3072
HCH = 24        # HID / P
EPS = 1e-5
SCALE = 0.125   # d ** -0.5

WNAMES = ['wq1', 'wk1', 'wv1', 'wp1', 'wq2', 'wkc', 'wvc', 'wp2',
          'wq4', 'wk4', 'wv4', 'wp4']
BNAMES = ['bq1', 'bk1', 'bv1', 'bp1', 'bq2', 'bp2',
          'bq4', 'bk4', 'bv4', 'bp4', 'bo']


# ----------------------------------------------------------------------------
# emission helpers
# ----------------------------------------------------------------------------

class Env:
    """Holds nc/tc, dram handles, const tiles, psum pools."""
    pass


def _ln_feat(env, x_slices, xh_out, sqpool, rowpool, bpool, tmppool, in_bf):
    """Feature-major LayerNorm (gamma/beta folded into downstream weights).

    Cross-partition sums run on GpSimd (partition_all_reduce broadcasts the
    sum to every partition), keeping the PE free.
    x_slices: 6 APs [128, Tq] (f32 or bf16); xh_out: 6 APs [128, Tq] bf16.
    """
    nc = env.nc
    Tq = x_slices[0].shape[1]
    xsum = sqpool.tile([P, 512], F32D, name="sq")[:, :Tq]
    nc.vector.tensor_add(xsum, x_slices[0], x_slices[1])
    for c in range(2, CH):
        nc.vector.tensor_add(xsum, xsum, x_slices[c])
    sqsum = sqpool.tile([P, 512], F32D, name="sq")[:, :Tq]
    nc.vector.tensor_mul(sqsum, x_slices[0], x_slices[0])
    for c in range(1, CH):
        sq = sqpool.tile([P, 512], F32D, name="sq")[:, :Tq]
        nc.vector.tensor_mul(sq, x_slices[c], x_slices[c])
        nc.vector.tensor_add(sqsum, sqsum, sq)
    bm = bpool.tile([P, 512], F32D, name="s1b")[:, :Tq]
    s2b = bpool.tile([P, 512], F32D, name="s2b")[:, :Tq]
    nc.gpsimd.partition_all_reduce(bm, xsum, channels=P,
                                   reduce_op=bass_isa.ReduceOp.add)
    nc.gpsimd.partition_all_reduce(s2b, sqsum, channels=P,
                                   reduce_op=bass_isa.ReduceOp.add)
    nc.vector.tensor_scalar_mul(bm, bm, 1.0 / C)   # bm = mean, all partitions
    m2row = rowpool.tile([1, 512], F32D, name="row")[:, :Tq]
    vrow = rowpool.tile([1, 512], F32D, name="row")[:, :Tq]
    srow = rowpool.tile([1, 512], F32D, name="row")[:, :Tq]
    rrow = rowpool.tile([1, 512], F32D, name="row")[:, :Tq]
    nc.vector.tensor_mul(m2row, bm[0:1, :], bm[0:1, :])
    nc.vector.tensor_scalar_mul(vrow, s2b[0:1, :], 1.0 / C)
    nc.vector.tensor_sub(vrow, vrow, m2row)
    nc.scalar.activation(srow, vrow, AF.Sqrt, bias=env.eps_tile[0:1, 0:1])
    nc.vector.reciprocal(rrow, srow)
    brs = bpool.tile([P, 512], F32D, name="brs")[:, :Tq]
    nc.gpsimd.partition_broadcast(brs, rrow)
    for c in range(CH):
        tmp = tmppool.tile([P, 512], F32D, name="lntmp")[:, :Tq]
        nc.vector.tensor_sub(tmp, x_slices[c], bm)
        nc.vector.tensor_mul(xh_out[c], tmp, brs)


def _proj_feat(env, wtiles, rhs_slices, bias_name, evict, nco=CH, Tq=T):
    """out^T[co-chunk] = sum_ci W[ci][:, co].T @ rhs[ci]  (+ bias row).

    evict(co, ps): consume psum tile [128, Tq]."""
    nc = env.nc
    bias = env.bias.get(bias_name)
    for co in range(nco):
        ps = env.psA.tile([P, 512], F32D, name="psA")[:, :Tq]
        for ci in range(len(rhs_slices)):
            nc.tensor.matmul(ps, wtiles[ci][:, co * P:(co + 1) * P], rhs_slices[ci],
                             start=(ci == 0),
                             stop=(ci == len(rhs_slices) - 1 and bias is None))
        if bias is not None:
            nc.tensor.matmul(ps, bias[0:1, co * P:(co + 1) * P],
                             env.ones_row[0:1, :Tq], start=False, stop=True)
        evict(co, ps)


def _v_aug(env, wtiles, xh_slices, bias_name, vpool, jcs, Tq=T, name="vaug"):
    """Token-major V with ones column per head: jcs tiles [128, 780] bf16."""
    nc = env.nc
    bias = env.bias.get(bias_name)
    vaug = []
    for jc in range(jcs):
        vt = vpool.tile([P, H * 65], BF16D, name=name if name == "vaug" else f"{name}{jc}")
        nc.vector.memset(vt[:], 1.0)
        vaug.append(vt)
    for jc in range(jcs):
        for cog in range(2):
            ps = env.psA.tile([P, 512], F32D, name="psA")[:, :384]
            for ci in range(CH):
                nc.tensor.matmul(
                    ps, xh_slices[ci][:, jc * P:(jc + 1) * P],
                    wtiles[ci][:, cog * 384:(cog + 1) * 384],
                    start=(ci == 0), stop=(ci == CH - 1 and bias is None))
            if bias is not None:
                nc.tensor.matmul(ps, env.ones_row[0:1, 0:P],
                                 bias[0:1, cog * 384:(cog + 1) * 384],
                                 start=False, stop=True)
            dst = vaug[jc][:, :].rearrange("p (h x) -> p h x", x=65)
            dst = dst[:, cog * 6:(cog + 1) * 6, 0:64]
            src = ps.rearrange("p (h x) -> p h x", x=64)
            nc.scalar.activation(dst, src, AF.Copy)
    return vaug


def _attn_full(env, qbig, kbig, vaug, avbig, etpool, rowpool, brdpool):
    """Phase-1/2 attention: full-T (512 keys, 4 j-chunks).

    Heads are processed in even/odd pairs: the K=64 sim matmuls of the pair
    run as concurrent 64x128 row-tiles (partitions 0-63 / 64-127), and all
    sims are issued before all avs so the PE tiling mode only switches twice
    per pair instead of per matmul."""
    nc = env.nc
    for hp in range(H // 2):
        co = hp
        ets = {0: [], 64: []}
        for jc in range(4):
            for po in (0, 64):
                qh = qbig[po:po + 64, co * T:(co + 1) * T]
                kh = kbig[po:po + 64, co * T:(co + 1) * T]
                ps = env.psS.tile([P, 512], F32D, name="psS")
                nc.tensor.matmul(ps, kh[:, jc * P:(jc + 1) * P], qh,
                                 start=True, stop=True)
                et = etpool.tile([P, 512], BF16D, name="et")
                nc.scalar.activation(et, ps, AF.Exp, scale=SCALE)
                ets[po].append(et)
        for po in (0, 64):
            h = 2 * hp + (1 if po else 0)
            av = env.psS.tile([P, 512], F32D, name="psS")
            for jc in range(4):
                nc.tensor.matmul(av[0:65, :], vaug[jc][:, 65 * h:65 * h + 65],
                                 ets[po][jc], start=(jc == 0), stop=(jc == 3))
            rr = rowpool.tile([1, 512], F32D, name="row")
            nc.vector.reciprocal(rr, av[64:65, :])
            brd = brdpool.tile([P, 512], F32D, name="brd")
            nc.gpsimd.partition_broadcast(brd, rr)
            dst = avbig[po:po + 64, co * T:(co + 1) * T]
            nc.scalar.activation(dst, av[0:64, :], AF.Copy)
            nc.vector.tensor_mul(dst, dst, brd[po:po + 64, :])


def _attn_grouped(env, qbig, kbig, vaug, avbig, etpool, rowpool, brdpool, ngroups):
    """Phase-3 frame attention: 128-col groups, block-diag mask applied to exp.

    Per group: all 12 heads' sims first (paired 64x128 row-tiles), then all
    avs, so the PE tiling mode switches twice per group."""
    nc = env.nc
    ncols = ngroups * P
    for g in range(ngroups):
        ets = []
        for h in range(H):
            po = 64 * (h % 2)
            co = h // 2
            qh = qbig[po:po + 64, co * ncols + g * P: co * ncols + (g + 1) * P]
            kh = kbig[po:po + 64, co * ncols + g * P: co * ncols + (g + 1) * P]
            ps = env.psS.tile([P, 512], F32D, name="psS")[:, :P]
            nc.tensor.matmul(ps, kh, qh, start=True, stop=True)
            et = etpool.tile([P, 512], BF16D, name="et")[:, :P]
            nc.scalar.activation(et, ps, AF.Exp, scale=SCALE)
            nc.vector.tensor_mul(et, et, env.mask)
            ets.append(et)
        for h in range(H):
            po = 64 * (h % 2)
            co = h // 2
            av = env.psS.tile([P, 512], F32D, name="psS")[:, :P]
            nc.tensor.matmul(av[0:65, :], vaug[g][:, 65 * h:65 * h + 65], ets[h],
                             start=True, stop=True)
            rr = rowpool.tile([1, 512], F32D, name="row")[:, :P]
            nc.vector.reciprocal(rr, av[64:65, :])
            brd = brdpool.tile([P, 512], F32D, name="brd")[:, :P]
            nc.gpsimd.partition_broadcast(brd, rr)
            dst = avbig[po:po + 64, co * ncols + g * P: co * ncols + (g + 1) * P]
            nc.scalar.activation(dst, av[0:64, :], AF.Copy)
            nc.vector.tensor_mul(dst, dst, brd[po:po + 64, :])


def _load_weight(env, pool, dram, nci, width):
    """One DMA for the whole weight: [nci*128, width] -> [128, nci*width]."""
    big = pool.tile([P, nci * width], BF16D, name=f"{dram.name}_w")
    dst = big[:, :].rearrange("p (c w) -> p c w", w=width)
    srcr = dram.ap().rearrange("(c p) w -> p c w", p=P)
    env.nc.sync.dma_start(dst, srcr)
    return [big[:, ci * width:(ci + 1) * width] for ci in range(nci)]


# ----------------------------------------------------------------------------
# main emission
# ----------------------------------------------------------------------------

def emit(ctx, tc, env, bias_flags, no_cc=False):
    nc = env.nc

    def _a2a(in_d, out_d):
        if no_cc:
            nc.sync.dma_start(out_d.ap()[:, :, :, :], in_d.ap()[:, :, :, :])
        else:
            nc.gpsimd.collective_compute(
                "AllToAll", mybir.AluOpType.bypass,
                replica_groups=[list(range(NCORES))],
                ins=[in_d.ap()[:, :, :, :]],
                outs=[out_d.ap()[:, :, :, :]])

    constp = ctx.enter_context(tc.tile_pool(name="const", bufs=1))
    env.ones_col_f32 = constp.tile([P, 1], F32D, name="ones_col_f32")
    nc.vector.memset(env.ones_col_f32[:], 1.0)
    env.ones_col_bf = constp.tile([P, 1], BF16D, name="ones_col_bf")
    nc.vector.memset(env.ones_col_bf[:], 1.0)
    env.ones_row = constp.tile([1, 512], F32D, name="ones_row")
    nc.vector.memset(env.ones_row[:], 1.0)
    env.eps_tile = constp.tile([1, 1], F32D, name="eps_tile")
    nc.vector.memset(env.eps_tile[:], EPS)
    env.mask = constp.tile([P, P], BF16D, name="maskt")
    nc.sync.dma_start(env.mask[:], env.d['mask01'].ap()[:, :])
    env.bias = {}
    for bn in BNAMES + ['bh']:
        if bias_flags.get(bn):
            width = HID if bn == 'bh' else C
            bt = constp.tile([1, width], F32D, name=f"{bn}_t")
            nc.sync.dma_start(bt[:], env.d[bn].ap()[:, :])
            env.bias[bn] = bt

    # PSUM budget: psA 1tag*3 + psS 1tag*4 = 7 banks
    env.psA = ctx.enter_context(tc.tile_pool(name="psA", bufs=3, space="PSUM"))
    env.psS = ctx.enter_context(tc.tile_pool(name="psS", bufs=4, space="PSUM"))

    # ---------------- phases 1 + 2 ----------------
    with ExitStack() as p12:
        sqpool = p12.enter_context(tc.tile_pool(name="sq", bufs=4))
        rowpool = p12.enter_context(tc.tile_pool(name="rows", bufs=7))
        bpool = p12.enter_context(tc.tile_pool(name="bcast", bufs=2))
        tmppool = p12.enter_context(tc.tile_pool(name="tmp", bufs=2))
        xhpool = p12.enter_context(tc.tile_pool(name="xh", bufs=9))
        qpool = p12.enter_context(tc.tile_pool(name="q", bufs=2))
        vpool = p12.enter_context(tc.tile_pool(name="vaug", bufs=6))
        etpool = p12.enter_context(tc.tile_pool(name="et", bufs=8))
        brdpool = p12.enter_context(tc.tile_pool(name="brd", bufs=4))
        avpool = p12.enter_context(tc.tile_pool(name="av", bufs=2))
        ypool = p12.enter_context(tc.tile_pool(name="ystage", bufs=4))
        # per-frame residual stream tiles; spilled to DRAM between the phases
        xfpool = p12.enter_context(tc.tile_pool(name="xf", bufs=2))

        # ---- phase 1: self-attention over T, per frame ----
        with ExitStack() as ph1:
            wself = ph1.enter_context(tc.tile_pool(name="wself", bufs=1))
            kpool = ph1.enter_context(tc.tile_pool(name="k", bufs=2))
            wq1 = _load_weight(env, wself, env.d['wq1'], CH, C)
            wk1 = _load_weight(env, wself, env.d['wk1'], CH, C)
            wv1 = _load_weight(env, wself, env.d['wv1'], CH, C)
            wp1 = _load_weight(env, wself, env.d['wp1'], CH, C)
            for f in range(NF):
                xfbig = xfpool.tile([P, CH * T], F32D, name="xf")
                nc.sync.dma_start(
                    xfbig[:, :].rearrange("p (c t) -> p c t", t=T),
                    env.d['xT'].ap()[f].rearrange("(c p) t -> p c t", p=P))
                xs = [xfbig[:, c * T:(c + 1) * T] for c in range(CH)]
                xh = []
                for c in range(CH):
                    t = xhpool.tile([P, T], BF16D, name="xh")
                    xh.append(t)
                _ln_feat(env, xs, xh, sqpool, rowpool, bpool, tmppool, in_bf=False)
                qbig = qpool.tile([P, CH * T], BF16D, name="qbig")
                kbig = kpool.tile([P, CH * T], BF16D, name="kbig")
                _proj_feat(env, wq1, xh, 'bq1',
                           lambda co, ps: nc.scalar.activation(
                               qbig[:, co * T:(co + 1) * T], ps, AF.Copy))
                _proj_feat(env, wk1, xh, 'bk1',
                           lambda co, ps: nc.scalar.activation(
                               kbig[:, co * T:(co + 1) * T], ps, AF.Copy))
                vaug = _v_aug(env, wv1, xh, 'bv1', vpool, 4)
                avbig = avpool.tile([P, CH * T], BF16D, name="avbig")
                _attn_full(env, qbig, kbig, vaug, avbig, etpool, rowpool, brdpool)
                avs = [avbig[:, c * T:(c + 1) * T] for c in range(CH)]

                def evict_y1(co, ps, xs=xs):
                    nc.vector.tensor_add(xs[co], xs[co], ps)

                _proj_feat(env, wp1, avs, 'bp1', evict_y1)
                nc.sync.dma_start(
                    env.d['y1buf'].ap()[f].rearrange("(c p) t -> p c t", p=P),
                    xfbig[:, :].rearrange("p (c t) -> p c t", t=T))

        # ---- phase 2: cross-attention, per frame ----
        with ExitStack() as ph2:
            wcross = ph2.enter_context(tc.tile_pool(name="wcross", bufs=1))
            wq2 = _load_weight(env, wcross, env.d['wq2'], CH, C)
            wkc = _load_weight(env, wcross, env.d['wkc'], CH, C)
            wvc = _load_weight(env, wcross, env.d['wvc'], CH, C)
            wp2 = _load_weight(env, wcross, env.d['wp2'], CH, C)
            condbig = wcross.tile([P, CH * T], BF16D, name="condbig")
            nc.sync.dma_start(
                condbig[:, :].rearrange("p (c t) -> p c t", t=T),
                env.d['condT'].ap().rearrange("(c p) t -> p c t", p=P))
            condb = [condbig[:, c * T:(c + 1) * T] for c in range(CH)]
            kcbig = wcross.tile([P, CH * T], BF16D, name="kcbig")
            _proj_feat(env, wkc, condb, None,
                       lambda co, ps: nc.scalar.activation(
                           kcbig[:, co * T:(co + 1) * T], ps, AF.Copy))
            vcaug = _v_aug(env, wvc, condb, None, wcross, 4, name="vc")

            for f in range(NF):
                xfbig = xfpool.tile([P, CH * T], F32D, name="xf")
                nc.sync.dma_start(
                    xfbig[:, :].rearrange("p (c t) -> p c t", t=T),
                    env.d['y1buf'].ap()[f].rearrange("(c p) t -> p c t", p=P))
                xs = [xfbig[:, c * T:(c + 1) * T] for c in range(CH)]
                xh = []
                for c in range(CH):
                    t = xhpool.tile([P, T], BF16D, name="xh")
                    xh.append(t)
                _ln_feat(env, xs, xh, sqpool, rowpool, bpool, tmppool, in_bf=False)
                qbig = qpool.tile([P, CH * T], BF16D, name="qbig")
                _proj_feat(env, wq2, xh, 'bq2',
                           lambda co, ps: nc.scalar.activation(
                               qbig[:, co * T:(co + 1) * T], ps, AF.Copy))
                avbig = avpool.tile([P, CH * T], BF16D, name="avbig")
                _attn_full(env, qbig, kcbig, vcaug, avbig, etpool, rowpool, brdpool)
                avs = [avbig[:, c * T:(c + 1) * T] for c in range(CH)]

                buf = env.d['in_bufA'] if f < 2 else env.d['in_bufB']
                fb = f % 2

                def evict_y2(co, ps, buf=buf, fb=fb, xs=xs):
                    yst = ypool.tile([P, T], BF16D, name="ystage")
                    nc.vector.tensor_add(yst, xs[co], ps)
                    dst = buf.ap()[:, fb, co * P:(co + 1) * P, :].transpose([1, 0, 2])
                    src = yst[:, :].rearrange("c (j t) -> c j t", j=NCORES)
                    nc.sync.dma_start(dst, src)

                _proj_feat(env, wp2, avs, 'bp2', evict_y2)

                if f == 1:
                    _a2a(env.d['in_bufA'], env.d['out_bufA'])
            _a2a(env.d['in_bufB'], env.d['out_bufB'])

    # ---------------- phases 3 + 4, per b-half ----------------
    NCOL = 1024   # columns per half: col = t*16 + n
    for hb in range(2):
        with ExitStack() as p34:
            zpool = p34.enter_context(tc.tile_pool(name="zpool", bufs=1))
            zbig = zpool.tile([P, CH * NCOL], BF16D, name="zbig")   # freed use after ph3
            # load frame-major (contiguous DMA), then repack to t-major on DVE:
            # zbig[p, t*16+n] = znbig[p, n*64+t]
            with ExitStack() as zl:
                znpool = zl.enter_context(tc.tile_pool(name="zn", bufs=1))
                znbig = znpool.tile([P, CH * NCOL], BF16D, name="znbig")
                for fb2, buf in ((0, env.d['out_bufA']), (1, env.d['out_bufB'])):
                    for f2 in range(2):
                        off = 2 * fb2 + f2     # n = 4*i4 + off
                        for c in range(CH):
                            srcp = buf.ap()[4 * hb:4 * hb + 4, f2,
                                            c * P:(c + 1) * P, :].transpose([1, 0, 2])
                            dstp = znbig[:, c * NCOL:(c + 1) * NCOL]
                            dstp = dstp.rearrange("p (i o t) -> p o i t",
                                                  i=4, o=4)[:, off]
                            nc.sync.dma_start(dstp, srcp)
                for c in range(CH):
                    dstp = zbig[:, c * NCOL:(c + 1) * NCOL]
                    dstp = dstp.rearrange("p (t n) -> p t n", n=16)
                    srcp = znbig[:, c * NCOL:(c + 1) * NCOL]
                    srcp = srcp.rearrange("p (n t) -> p n t", t=TLOC)
                    nc.vector.tensor_copy(dstp, srcp.transpose([0, 2, 1]))

            sqpool = p34.enter_context(tc.tile_pool(name="sq3", bufs=4))
            rowpool = p34.enter_context(tc.tile_pool(name="rows3", bufs=7))
            bpool = p34.enter_context(tc.tile_pool(name="bcast3", bufs=2))
            tmppool = p34.enter_context(tc.tile_pool(name="tmp3", bufs=2))
            xhpool = p34.enter_context(tc.tile_pool(name="xh3", bufs=8))
            etpool = p34.enter_context(tc.tile_pool(name="et3", bufs=14))
            brdpool = p34.enter_context(tc.tile_pool(name="brd3", bufs=4))
            z3pool = p34.enter_context(tc.tile_pool(name="z3", bufs=1))
            z3big = z3pool.tile([P, CH * NCOL], BF16D, name="z3big")

            with ExitStack() as ph3:
                w4p = ph3.enter_context(tc.tile_pool(name="w4", bufs=1))
                wq4 = _load_weight(env, w4p, env.d['wq4'], CH, C)
                wk4 = _load_weight(env, w4p, env.d['wk4'], CH, C)
                wv4 = _load_weight(env, w4p, env.d['wv4'], CH, C)
                wp4 = _load_weight(env, w4p, env.d['wp4'], CH, C)
                qk4p = ph3.enter_context(tc.tile_pool(name="qk4", bufs=1))
                q4big = qk4p.tile([P, CH * NCOL], BF16D, name="q4big")
                k4big = qk4p.tile([P, CH * NCOL], BF16D, name="k4big")
                v4p = ph3.enter_context(tc.tile_pool(name="v4", bufs=8))
                av4p = ph3.enter_context(tc.tile_pool(name="av4", bufs=1))
                av4big = av4p.tile([P, CH * NCOL], BF16D, name="av4big")
                vaug4 = [None] * 8
                for cb in range(2):
                    cs = slice(cb * T, (cb + 1) * T)
                    zs = [zbig[:, c * NCOL:(c + 1) * NCOL][:, cs] for c in range(CH)]
                    xh4 = []
                    for c in range(CH):
                        t = xhpool.tile([P, T], BF16D, name="xh4")
                        xh4.append(t)
                    _ln_feat(env, zs, xh4, sqpool, rowpool, bpool, tmppool, in_bf=True)
                    _proj_feat(env, wq4, xh4, 'bq4',
                               lambda co, ps, cb=cb: nc.scalar.activation(
                                   q4big[:, co * NCOL + cb * T:co * NCOL + (cb + 1) * T],
                                   ps, AF.Copy))
                    _proj_feat(env, wk4, xh4, 'bk4',
                               lambda co, ps, cb=cb: nc.scalar.activation(
                                   k4big[:, co * NCOL + cb * T:co * NCOL + (cb + 1) * T],
                                   ps, AF.Copy))
                    vg = _v_aug(env, wv4, xh4, 'bv4', v4p, 4)
                    for g in range(4):
                        vaug4[cb * 4 + g] = vg[g]
                _attn_grouped(env, q4big, k4big, vaug4, av4big, etpool, rowpool,
                              brdpool, ngroups=8)
                for cb in range(2):
                    avs = [av4big[:, c * NCOL + cb * T:c * NCOL + (cb + 1) * T]
                           for c in range(CH)]

                    def evict_z3(co, ps, cb=cb):
                        dst = z3big[:, co * NCOL + cb * T:co * NCOL + (cb + 1) * T]
                        nc.vector.tensor_add(
                            dst, zbig[:, co * NCOL + cb * T:co * NCOL + (cb + 1) * T], ps)

                    _proj_feat(env, wp4, avs, 'bp4', evict_z3)

            # ---- phase 4: MLP ----
            with ExitStack() as ph4:
                wmp = ph4.enter_context(tc.tile_pool(name="wm", bufs=1))
                w1t = _load_weight(env, wmp, env.d['w1'], CH, HID)
                w2t = _load_weight(env, wmp, env.d['w2'], HCH, C)
                h1pool = ph4.enter_context(tc.tile_pool(name="h1", bufs=1))
                opool = ph4.enter_context(tc.tile_pool(name="ostage", bufs=1))
                bh = env.bias.get('bh')
                bo = env.bias.get('bo')
                for cb in range(2):
                    obig = opool.tile([P, CH * T], F32D, name="obig")
                    z3s = [z3big[:, c * NCOL + cb * T:c * NCOL + (cb + 1) * T]
                           for c in range(CH)]
                    xh3 = []
                    for c in range(CH):
                        t = xhpool.tile([P, T], BF16D, name="xh4")
                        xh3.append(t)
                    _ln_feat(env, z3s, xh3, sqpool, rowpool, bpool, tmppool, in_bf=True)
                    h1big = h1pool.tile([P, HCH * T], BF16D, name="h1big")
                    for co in range(HCH):
                        ps = env.psA.tile([P, 512], F32D, name="psA")
                        for ci in range(CH):
                            nc.tensor.matmul(ps, w1t[ci][:, co * P:(co + 1) * P],
                                             xh3[ci], start=(ci == 0),
                                             stop=(ci == CH - 1 and bh is None))
                        if bh is not None:
                            nc.tensor.matmul(ps, bh[0:1, co * P:(co + 1) * P],
                                             env.ones_row[0:1, :], start=False, stop=True)
                        nc.scalar.activation(h1big[:, co * T:(co + 1) * T], ps, AF.Gelu)
                    for co in range(CH):
                        ps = env.psA.tile([P, 512], F32D, name="psA")
                        for ci in range(HCH):
                            nc.tensor.matmul(ps, w2t[ci][:, co * P:(co + 1) * P],
                                             h1big[:, ci * T:(ci + 1) * T],
                                             start=(ci == 0),
                                             stop=(ci == HCH - 1 and bo is None))
                        if bo is not None:
                            nc.tensor.matmul(ps, bo[0:1, co * P:(co + 1) * P],
                                             env.ones_row[0:1, :], start=False, stop=True)
                        nc.vector.tensor_add(
                            obig[:, co * T:(co + 1) * T],
                            z3big[:, co * NCOL + cb * T:co * NCOL + (cb + 1) * T], ps)
                    nc.sync.dma_start(
                        env.d['out'].ap()[:, hb * NCOL + cb * T:hb * NCOL + (cb + 1) * T]
                           .rearrange("(c p) t -> p c t", p=P),
                        obig[:, :].rearrange("p (c t) -> p c t", t=T))


# ----------------------------------------------------------------------------
# program build + host glue
# ----------------------------------------------------------------------------

_CACHE = {}


def build_program(bias_flags_key, no_cc=False, rounds=1):
    cache_key = (bias_flags_key, no_cc, rounds)
    if cache_key in _CACHE:
        return _CACHE[cache_key]
    bias_flags = dict(bias_flags_key)
    nc = bacc.Bacc("TRN2", target_bir_lowering=False, debug=False,
                   num_devices=NCORES)
    env = Env()
    env.nc = nc
    d = {}
    d['xT'] = nc.dram_tensor("xT", [NF, C, T], F32D, kind="ExternalInput")
    d['condT'] = nc.dram_tensor("condT", [C, T], BF16D, kind="ExternalInput")
    for w in WNAMES:
        d[w] = nc.dram_tensor(w, [C, C], BF16D, kind="ExternalInput")
    d['w1'] = nc.dram_tensor("w1", [C, HID], BF16D, kind="ExternalInput")
    d['w2'] = nc.dram_tensor("w2", [HID, C], BF16D, kind="ExternalInput")
    for bn in BNAMES:
        if bias_flags.get(bn):
            d[bn] = nc.dram_tensor(bn, [1, C], F32D, kind="ExternalInput")
    if bias_flags.get('bh'):
        d['bh'] = nc.dram_tensor("bh", [1, HID], F32D, kind="ExternalInput")
    d['mask01'] = nc.dram_tensor("mask01", [P, P], BF16D, kind="ExternalInput")
    d['out'] = nc.dram_tensor("out", [C, 2048], F32D, kind="ExternalOutput")
    for nm in ('in_bufA', 'in_bufB', 'out_bufA', 'out_bufB'):
        d[nm] = nc.dram_tensor(nm, [NCORES, 2, C, TLOC], BF16D)
    d['y1buf'] = nc.dram_tensor("y1buf", [NF, C, T], F32D)
    env.d = d

    with tile.TileContext(nc) as tc:
        for _ in range(rounds):
            with ExitStack() as ctx:
                emit(ctx, tc, env, bias_flags, no_cc=no_cc)
    nc.compile()
    _CACHE[cache_key] = nc
    return nc


NCOL_OUT = 1024


def prep_inputs(x, cond, params):
    """Host-side prep: gamma folding, transposes, per-core sharding."""
    x = np.asarray(x, f32)
    cond = np.asarray(cond, f32)
    g1, be1 = [np.asarray(a, f32) for a in params['ln1']]
    g2, be2 = [np.asarray(a, f32) for a in params['ln2']]
    g3, be3 = [np.asarray(a, f32) for a in params['ln3']]
    g4, be4 = [np.asarray(a, f32) for a in params['ln4']]

    def fold(w, g):
        return np.ascontiguousarray((np.asarray(w, f32) * g[None, :]).T).astype(BF)

    def plain(w):
        return np.ascontiguousarray(np.asarray(w, f32).T).astype(BF)

    def lnbias(w, b):
        return (np.asarray(w, f32) @ b).astype(f32)

    at, cr, a4, ml = params['attn'], params['cross'], params['attn_t'], params['mlp']
    W = {
        'wq1': fold(at['wq'], g1), 'wk1': fold(at['wk'], g1),
        'wv1': fold(at['wv'], g1), 'wp1': plain(at['wp']),
        'wq2': fold(cr['wq'], g2), 'wkc': plain(cr['wk']),
        'wvc': plain(cr['wv']), 'wp2': plain(cr['wp']),
        'wq4': fold(a4['wq'], g4), 'wk4': fold(a4['wk'], g4),
        'wv4': fold(a4['wv'], g4), 'wp4': plain(a4['wp']),
        'w1': fold(ml['w1'], g3), 'w2': plain(ml['w2']),
    }
    Bv = {
        'bq1': lnbias(at['wq'], be1), 'bk1': lnbias(at['wk'], be1),
        'bv1': lnbias(at['wv'], be1), 'bp1': np.asarray(at['bp'], f32),
        'bq2': lnbias(cr['wq'], be2), 'bp2': np.asarray(cr['bp'], f32),
        'bq4': lnbias(a4['wq'], be4), 'bk4': lnbias(a4['wk'], be4),
        'bv4': lnbias(a4['wv'], be4), 'bp4': np.asarray(a4['bp'], f32),
        'bh': (lnbias(ml['w1'], be3) + np.asarray(ml['b1'], f32)),
        'bo': np.asarray(ml['b2'], f32),
    }
    bias_flags = {k: bool(np.any(v != 0)) for k, v in Bv.items()}

    mask01 = np.zeros((P, P), BF)
    for blk in range(8):
        mask01[blk * 16:(blk + 1) * 16, blk * 16:(blk + 1) * 16] = 1

    in_maps = []
    for i in range(NCORES):
        b, s = i // 4, i % 4
        m = {
            'xT': np.ascontiguousarray(x[b, 4 * s:4 * s + 4].transpose(0, 2, 1)),
            'condT': np.ascontiguousarray(cond[b].T).astype(BF),
            'mask01': mask01,
        }
        m.update(W)
        for k, fl in bias_flags.items():
            if fl:
                m[k] = np.ascontiguousarray(Bv[k][None, :])
        in_maps.append(m)
    return in_maps, bias_flags


def assemble_output(results):
    """results: list of per-core dicts with 'out' [768, 2048] f32."""
    B, N, Tfull = 2, 16, 512
    out = np.zeros((B, N, Tfull, C), f32)
    for j in range(NCORES):
        o = results[j]['out']   # [C, 2048]
        oz = o.reshape(C, 2, TLOC, 16).transpose(1, 3, 2, 0)
        out[:, :, TLOC * j:TLOC * (j + 1), :] = oz
    return out


LAST_RESULTS = None


def kernel(x, cond, params):
    global LAST_RESULTS
    in_maps, bias_flags = prep_inputs(x, cond, params)
    nc = build_program(tuple(sorted(bias_flags.items())))
    res = run_bass_kernel_spmd(nc, in_maps, list(range(NCORES)))
    LAST_RESULTS = res
    return assemble_output(res.results)


if __name__ == "__main__":
    import reference
    inputs = reference.setup_inputs()
    out = kernel(**{k: v for k, v in inputs.items()})
    print("kernel ran; out shape", out.shape)


# revision 28
# speedup vs baseline: 1.0693x; 1.0693x over previous
"""Trainium2 Bass kernel for nn_Block_49220325212407 (dense transformer block).

Strategy (8 NeuronCores, SPMD single NEFF):
- Phases 1/2 (self-attn over T, cross-attn): data-parallel over the 32
  (b, n) frames -> 4 frames per core.
- One 8-way AllToAll (split in two halves for overlap) reshards to
  (b, t-chunk) ownership: core j owns t-rows [64j, 64j+64) of all frames.
- Phase 3 (frame attention over N=16): block-diagonal batched attention,
  8 t-rows per 128-col group, multiplicative 0/1 mask after exp.
- Phase 4 (MLP): per-token, computed in the resharded layout.

All activations live feature-major ([C, tokens] on SBUF partitions), so no
transposes are ever needed: LN stats via ones-column matmuls, broadcasts via
GpSimd partition_broadcast, softmax denominators via a ones-column appended
to V (v_aug), and the av matmul produces feature-major output directly.
Matmuls run in bf16 with f32 PSUM accumulation; the residual stream stays
f32 through phase 1 and is rounded to bf16 at the AllToAll and after
phase 3 (rel err ~6e-3 vs the f32 reference).
"""
import os
from contextlib import ExitStack

import numpy as np
import ml_dtypes

import concourse.bass as bass
import concourse.bass_isa as bass_isa
import concourse.tile as tile
from concourse import bacc, mybir
from concourse.bass_utils import run_bass_kernel_spmd

f32 = np.float32
BF = ml_dtypes.bfloat16
F32D = mybir.dt.float32
BF16D = mybir.dt.bfloat16
AF = mybir.ActivationFunctionType

P = 128
C = 768
CH = 6          # C / P
T = 512
H = 12
D = 64
NF = 4          # frames per core
NCORES = 8
TLOC = T // NCORES   # 64
HID = 3072
HCH = 24        # HID / P
EPS = 1e-5
SCALE = 0.125   # d ** -0.5

WNAMES = ['wq1', 'wk1', 'wv1', 'wp1', 'wq2', 'wkc', 'wvc', 'wp2',
          'wq4', 'wk4', 'wv4', 'wp4']
BNAMES = ['bq1', 'bk1', 'bv1', 'bp1', 'bq2', 'bp2',
          'bq4', 'bk4', 'bv4', 'bp4', 'bo']


# ----------------------------------------------------------------------------
# emission helpers
# ----------------------------------------------------------------------------

class Env:
    """Holds nc/tc, dram handles, const tiles, psum pools."""
    pass


def _ln_feat(env, x_slices, xh_out, sqpool, rowpool, bpool, tmppool, in_bf):
    """Feature-major LayerNorm (gamma/beta folded into downstream weights).

    Cross-partition sums run on GpSimd (partition_all_reduce broadcasts the
    sum to every partition), keeping the PE free.
    x_slices: 6 APs [128, Tq] (f32 or bf16); xh_out: 6 APs [128, Tq] bf16.
    """
    nc = env.nc
    Tq = x_slices[0].shape[1]
    xsum = sqpool.tile([P, 512], F32D, name="sq")[:, :Tq]
    nc.vector.tensor_add(xsum, x_slices[0], x_slices[1])
    for c in range(2, CH):
        nc.vector.tensor_add(xsum, xsum, x_slices[c])
    sqsum = sqpool.tile([P, 512], F32D, name="sq")[:, :Tq]
    nc.vector.tensor_mul(sqsum, x_slices[0], x_slices[0])
    for c in range(1, CH):
        sq = sqpool.tile([P, 512], F32D, name="sq")[:, :Tq]
        nc.vector.tensor_mul(sq, x_slices[c], x_slices[c])
        nc.vector.tensor_add(sqsum, sqsum, sq)
    bm = bpool.tile([P, 512], F32D, name="s1b")[:, :Tq]
    s2b = bpool.tile([P, 512], F32D, name="s2b")[:, :Tq]
    nc.gpsimd.partition_all_reduce(bm, xsum, channels=P,
                                   reduce_op=bass_isa.ReduceOp.add)
    nc.gpsimd.partition_all_reduce(s2b, sqsum, channels=P,
                                   reduce_op=bass_isa.ReduceOp.add)
    nc.vector.tensor_scalar_mul(bm, bm, 1.0 / C)   # bm = mean, all partitions
    m2row = rowpool.tile([1, 512], F32D, name="row")[:, :Tq]
    vrow = rowpool.tile([1, 512], F32D, name="row")[:, :Tq]
    srow = rowpool.tile([1, 512], F32D, name="row")[:, :Tq]
    rrow = rowpool.tile([1, 512], F32D, name="row")[:, :Tq]
    nc.vector.tensor_mul(m2row, bm[0:1, :], bm[0:1, :])
    nc.vector.tensor_scalar_mul(vrow, s2b[0:1, :], 1.0 / C)
    nc.vector.tensor_sub(vrow, vrow, m2row)
    nc.scalar.activation(srow, vrow, AF.Sqrt, bias=env.eps_tile[0:1, 0:1])
    nc.vector.reciprocal(rrow, srow)
    brs = bpool.tile([P, 512], F32D, name="brs")[:, :Tq]
    nc.gpsimd.partition_broadcast(brs, rrow)
    for c in range(CH):
        tmp = tmppool.tile([P, 512], F32D, name="lntmp")[:, :Tq]
        nc.vector.tensor_sub(tmp, x_slices[c], bm)
        nc.vector.tensor_mul(xh_out[c], tmp, brs)


def _proj_feat(env, wtiles, rhs_slices, bias_name, evict, nco=CH, Tq=T):
    """out^T[co-chunk] = sum_ci W[ci][:, co].T @ rhs[ci]  (+ bias row).

    evict(co, ps): consume psum tile [128, Tq]."""
    nc = env.nc
    bias = env.bias.get(bias_name)
    for co in range(nco):
        ps = env.psA.tile([P, 512], F32D, name="psA")[:, :Tq]
        for ci in range(len(rhs_slices)):
            nc.tensor.matmul(ps, wtiles[ci][:, co * P:(co + 1) * P], rhs_slices[ci],
                             start=(ci == 0),
                             stop=(ci == len(rhs_slices) - 1 and bias is None))
        if bias is not None:
            nc.tensor.matmul(ps, bias[0:1, co * P:(co + 1) * P],
                             env.ones_row[0:1, :Tq], start=False, stop=True)
        evict(co, ps)


def _v_aug(env, wtiles, xh_slices, bias_name, vpool, jcs, Tq=T, name="vaug"):
    """Token-major V with ones column per head: jcs tiles [128, 780] bf16."""
    nc = env.nc
    bias = env.bias.get(bias_name)
    vaug = []
    for jc in range(jcs):
        vt = vpool.tile([P, H * 65], BF16D, name=name if name == "vaug" else f"{name}{jc}")
        nc.vector.memset(vt[:], 1.0)
        vaug.append(vt)
    for jc in range(jcs):
        for cog in range(2):
            ps = env.psA.tile([P, 512], F32D, name="psA")[:, :384]
            for ci in range(CH):
                nc.tensor.matmul(
                    ps, xh_slices[ci][:, jc * P:(jc + 1) * P],
                    wtiles[ci][:, cog * 384:(cog + 1) * 384],
                    start=(ci == 0), stop=(ci == CH - 1 and bias is None))
            if bias is not None:
                nc.tensor.matmul(ps, env.ones_row[0:1, 0:P],
                                 bias[0:1, cog * 384:(cog + 1) * 384],
                                 start=False, stop=True)
            dst = vaug[jc][:, :].rearrange("p (h x) -> p h x", x=65)
            dst = dst[:, cog * 6:(cog + 1) * 6, 0:64]
            src = ps.rearrange("p (h x) -> p h x", x=64)
            nc.scalar.activation(dst, src, AF.Copy)
    return vaug


def _attn_full(env, qbig, kbig, vaug, avbig, etpool, rowpool, brdpool):
    """Phase-1/2 attention: full-T (512 keys, 4 j-chunks).

    Heads are processed in even/odd pairs: the K=64 sim matmuls of the pair
    run as concurrent 64x128 row-tiles (partitions 0-63 / 64-127), and all
    sims are issued before all avs so the PE tiling mode only switches twice
    per pair instead of per matmul."""
    nc = env.nc
    for hq in range(H // 4):   # 4 heads per batch: 2 pairs
        ets = {}
        for hpi in (0, 1):
            hp = 2 * hq + hpi
            co = hp
            ets[hpi] = {0: [], 64: []}
            for jc in range(4):
                for po in (0, 64):
                    qh = qbig[po:po + 64, co * T:(co + 1) * T]
                    kh = kbig[po:po + 64, co * T:(co + 1) * T]
                    ps = env.psS.tile([P, 512], F32D, name="psS")
                    nc.tensor.matmul(ps, kh[:, jc * P:(jc + 1) * P], qh,
                                     start=True, stop=True)
                    et = etpool.tile([P, 512], BF16D, name="et")
                    nc.scalar.activation(et, ps, AF.Exp, scale=SCALE)
                    ets[hpi][po].append(et)
        for hpi in (0, 1):
            hp = 2 * hq + hpi
            co = hp
            for po in (0, 64):
                h = 2 * hp + (1 if po else 0)
                av = env.psS.tile([P, 512], F32D, name="psS")
                for jc in range(4):
                    nc.tensor.matmul(av[0:65, :], vaug[jc][:, 65 * h:65 * h + 65],
                                     ets[hpi][po][jc], start=(jc == 0), stop=(jc == 3))
                rr = rowpool.tile([1, 512], F32D, name="row")
                nc.vector.reciprocal(rr, av[64:65, :])
                brd = brdpool.tile([P, 512], F32D, name="brd")
                nc.gpsimd.partition_broadcast(brd, rr)
                dst = avbig[po:po + 64, co * T:(co + 1) * T]
                nc.scalar.activation(dst, av[0:64, :], AF.Copy)
                nc.vector.tensor_mul(dst, dst, brd[po:po + 64, :])


def _attn_grouped(env, qbig, kbig, vaug, avbig, etpool, rowpool, brdpool, ngroups):
    """Phase-3 frame attention: 128-col groups, block-diag mask applied to exp.

    Per group: all 12 heads' sims first (paired 64x128 row-tiles), then all
    avs, so the PE tiling mode switches twice per group."""
    nc = env.nc
    ncols = ngroups * P
    for g in range(ngroups):
        ets = []
        for h in range(H):
            po = 64 * (h % 2)
            co = h // 2
            qh = qbig[po:po + 64, co * ncols + g * P: co * ncols + (g + 1) * P]
            kh = kbig[po:po + 64, co * ncols + g * P: co * ncols + (g + 1) * P]
            ps = env.psS.tile([P, 512], F32D, name="psS")[:, :P]
            nc.tensor.matmul(ps, kh, qh, start=True, stop=True)
            et = etpool.tile([P, 512], BF16D, name="et")[:, :P]
            nc.scalar.activation(et, ps, AF.Exp, scale=SCALE)
            nc.vector.tensor_mul(et, et, env.mask)
            ets.append(et)
        for h in range(H):
            po = 64 * (h % 2)
            co = h // 2
            av = env.psS.tile([P, 512], F32D, name="psS")[:, :P]
            nc.tensor.matmul(av[0:65, :], vaug[g][:, 65 * h:65 * h + 65], ets[h],
                             start=True, stop=True)
            rr = rowpool.tile([1, 512], F32D, name="row")[:, :P]
            nc.vector.reciprocal(rr, av[64:65, :])
            brd = brdpool.tile([P, 512], F32D, name="brd")[:, :P]
            nc.gpsimd.partition_broadcast(brd, rr)
            dst = avbig[po:po + 64, co * ncols + g * P: co * ncols + (g + 1) * P]
            nc.scalar.activation(dst, av[0:64, :], AF.Copy)
            nc.vector.tensor_mul(dst, dst, brd[po:po + 64, :])


def _load_weight(env, pool, dram, nci, width):
    """One DMA for the whole weight: [nci*128, width] -> [128, nci*width]."""
    big = pool.tile([P, nci * width], BF16D, name=f"{dram.name}_w")
    dst = big[:, :].rearrange("p (c w) -> p c w", w=width)
    srcr = dram.ap().rearrange("(c p) w -> p c w", p=P)
    env.nc.sync.dma_start(dst, srcr)
    return [big[:, ci * width:(ci + 1) * width] for ci in range(nci)]


# ----------------------------------------------------------------------------
# main emission
# ----------------------------------------------------------------------------

def emit(ctx, tc, env, bias_flags, no_cc=False):
    nc = env.nc

    def _a2a(in_d, out_d):
        if no_cc:
            nc.sync.dma_start(out_d.ap()[:, :, :, :], in_d.ap()[:, :, :, :])
        else:
            nc.gpsimd.collective_compute(
                "AllToAll", mybir.AluOpType.bypass,
                replica_groups=[list(range(NCORES))],
                ins=[in_d.ap()[:, :, :, :]],
                outs=[out_d.ap()[:, :, :, :]])

    constp = ctx.enter_context(tc.tile_pool(name="const", bufs=1))
    env.ones_col_f32 = constp.tile([P, 1], F32D, name="ones_col_f32")
    nc.vector.memset(env.ones_col_f32[:], 1.0)
    env.ones_col_bf = constp.tile([P, 1], BF16D, name="ones_col_bf")
    nc.vector.memset(env.ones_col_bf[:], 1.0)
    env.ones_row = constp.tile([1, 512], F32D, name="ones_row")
    nc.vector.memset(env.ones_row[:], 1.0)
    env.eps_tile = constp.tile([1, 1], F32D, name="eps_tile")
    nc.vector.memset(env.eps_tile[:], EPS)
    env.mask = constp.tile([P, P], BF16D, name="maskt")
    nc.sync.dma_start(env.mask[:], env.d['mask01'].ap()[:, :])
    env.bias = {}
    for bn in BNAMES + ['bh']:
        if bias_flags.get(bn):
            width = HID if bn == 'bh' else C
            bt = constp.tile([1, width], F32D, name=f"{bn}_t")
            nc.sync.dma_start(bt[:], env.d[bn].ap()[:, :])
            env.bias[bn] = bt

    # PSUM budget: psA 1tag*3 + psS 1tag*4 = 7 banks
    env.psA = ctx.enter_context(tc.tile_pool(name="psA", bufs=3, space="PSUM"))
    env.psS = ctx.enter_context(tc.tile_pool(name="psS", bufs=5, space="PSUM"))

    # ---------------- phases 1 + 2 ----------------
    with ExitStack() as p12:
        sqpool = p12.enter_context(tc.tile_pool(name="sq", bufs=4))
        rowpool = p12.enter_context(tc.tile_pool(name="rows", bufs=7))
        bpool = p12.enter_context(tc.tile_pool(name="bcast", bufs=2))
        tmppool = p12.enter_context(tc.tile_pool(name="tmp", bufs=2))
        xhpool = p12.enter_context(tc.tile_pool(name="xh", bufs=9))
        qpool = p12.enter_context(tc.tile_pool(name="q", bufs=2))
        vpool = p12.enter_context(tc.tile_pool(name="vaug", bufs=6))
        etpool = p12.enter_context(tc.tile_pool(name="et", bufs=16))
        brdpool = p12.enter_context(tc.tile_pool(name="brd", bufs=4))
        avpool = p12.enter_context(tc.tile_pool(name="av", bufs=2))
        ypool = p12.enter_context(tc.tile_pool(name="ystage", bufs=4))
        # per-frame residual stream tiles; spilled to DRAM between the phases
        xfpool = p12.enter_context(tc.tile_pool(name="xf", bufs=2))

        # ---- phase 1: self-attention over T, per frame ----
        with ExitStack() as ph1:
            wself = ph1.enter_context(tc.tile_pool(name="wself", bufs=1))
            kpool = ph1.enter_context(tc.tile_pool(name="k", bufs=2))
            wq1 = _load_weight(env, wself, env.d['wq1'], CH, C)
            wk1 = _load_weight(env, wself, env.d['wk1'], CH, C)
            wv1 = _load_weight(env, wself, env.d['wv1'], CH, C)
            wp1 = _load_weight(env, wself, env.d['wp1'], CH, C)
            for f in range(NF):
                xfbig = xfpool.tile([P, CH * T], F32D, name="xf")
                nc.sync.dma_start(
                    xfbig[:, :].rearrange("p (c t) -> p c t", t=T),
                    env.d['xT'].ap()[f].rearrange("(c p) t -> p c t", p=P))
                xs = [xfbig[:, c * T:(c + 1) * T] for c in range(CH)]
                xh = []
                for c in range(CH):
                    t = xhpool.tile([P, T], BF16D, name="xh")
                    xh.append(t)
                _ln_feat(env, xs, xh, sqpool, rowpool, bpool, tmppool, in_bf=False)
                qbig = qpool.tile([P, CH * T], BF16D, name="qbig")
                kbig = kpool.tile([P, CH * T], BF16D, name="kbig")
                _proj_feat(env, wq1, xh, 'bq1',
                           lambda co, ps: nc.scalar.activation(
                               qbig[:, co * T:(co + 1) * T], ps, AF.Copy))
                _proj_feat(env, wk1, xh, 'bk1',
                           lambda co, ps: nc.scalar.activation(
                               kbig[:, co * T:(co + 1) * T], ps, AF.Copy))
                vaug = _v_aug(env, wv1, xh, 'bv1', vpool, 4)
                avbig = avpool.tile([P, CH * T], BF16D, name="avbig")
                _attn_full(env, qbig, kbig, vaug, avbig, etpool, rowpool, brdpool)
                avs = [avbig[:, c * T:(c + 1) * T] for c in range(CH)]

                def evict_y1(co, ps, xs=xs):
                    nc.vector.tensor_add(xs[co], xs[co], ps)

                _proj_feat(env, wp1, avs, 'bp1', evict_y1)
                nc.sync.dma_start(
                    env.d['y1buf'].ap()[f].rearrange("(c p) t -> p c t", p=P),
                    xfbig[:, :].rearrange("p (c t) -> p c t", t=T))

        # ---- phase 2: cross-attention, per frame ----
        with ExitStack() as ph2:
            wcross = ph2.enter_context(tc.tile_pool(name="wcross", bufs=1))
            wq2 = _load_weight(env, wcross, env.d['wq2'], CH, C)
            wkc = _load_weight(env, wcross, env.d['wkc'], CH, C)
            wvc = _load_weight(env, wcross, env.d['wvc'], CH, C)
            wp2 = _load_weight(env, wcross, env.d['wp2'], CH, C)
            condbig = wcross.tile([P, CH * T], BF16D, name="condbig")
            nc.sync.dma_start(
                condbig[:, :].rearrange("p (c t) -> p c t", t=T),
                env.d['condT'].ap().rearrange("(c p) t -> p c t", p=P))
            condb = [condbig[:, c * T:(c + 1) * T] for c in range(CH)]
            kcbig = wcross.tile([P, CH * T], BF16D, name="kcbig")
            _proj_feat(env, wkc, condb, None,
                       lambda co, ps: nc.scalar.activation(
                           kcbig[:, co * T:(co + 1) * T], ps, AF.Copy))
            vcaug = _v_aug(env, wvc, condb, None, wcross, 4, name="vc")

            for f in range(NF):
                xfbig = xfpool.tile([P, CH * T], F32D, name="xf")
                nc.sync.dma_start(
                    xfbig[:, :].rearrange("p (c t) -> p c t", t=T),
                    env.d['y1buf'].ap()[f].rearrange("(c p) t -> p c t", p=P))
                xs = [xfbig[:, c * T:(c + 1) * T] for c in range(CH)]
                xh = []
                for c in range(CH):
                    t = xhpool.tile([P, T], BF16D, name="xh")
                    xh.append(t)
                _ln_feat(env, xs, xh, sqpool, rowpool, bpool, tmppool, in_bf=False)
                qbig = qpool.tile([P, CH * T], BF16D, name="qbig")
                _proj_feat(env, wq2, xh, 'bq2',
                           lambda co, ps: nc.scalar.activation(
                               qbig[:, co * T:(co + 1) * T], ps, AF.Copy))
                avbig = avpool.tile([P, CH * T], BF16D, name="avbig")
                _attn_full(env, qbig, kcbig, vcaug, avbig, etpool, rowpool, brdpool)
                avs = [avbig[:, c * T:(c + 1) * T] for c in range(CH)]

                buf = env.d['in_bufA'] if f < 2 else env.d['in_bufB']
                fb = f % 2

                def evict_y2(co, ps, buf=buf, fb=fb, xs=xs):
                    yst = ypool.tile([P, T], BF16D, name="ystage")
                    nc.vector.tensor_add(yst, xs[co], ps)
                    dst = buf.ap()[:, fb, co * P:(co + 1) * P, :].transpose([1, 0, 2])
                    src = yst[:, :].rearrange("c (j t) -> c j t", j=NCORES)
                    nc.sync.dma_start(dst, src)

                _proj_feat(env, wp2, avs, 'bp2', evict_y2)

                if f == 1:
                    _a2a(env.d['in_bufA'], env.d['out_bufA'])
            _a2a(env.d['in_bufB'], env.d['out_bufB'])

    # ---------------- phases 3 + 4, per b-half ----------------
    NCOL = 1024   # columns per half: col = t*16 + n
    for hb in range(2):
        with ExitStack() as p34:
            zpool = p34.enter_context(tc.tile_pool(name="zpool", bufs=1))
            zbig = zpool.tile([P, CH * NCOL], BF16D, name="zbig")   # freed use after ph3
            # load frame-major (contiguous DMA), then repack to t-major on DVE:
            # zbig[p, t*16+n] = znbig[p, n*64+t]
            with ExitStack() as zl:
                znpool = zl.enter_context(tc.tile_pool(name="zn", bufs=1))
                znbig = znpool.tile([P, CH * NCOL], BF16D, name="znbig")
                for fb2, buf in ((0, env.d['out_bufA']), (1, env.d['out_bufB'])):
                    for f2 in range(2):
                        off = 2 * fb2 + f2     # n = 4*i4 + off
                        for c in range(CH):
                            srcp = buf.ap()[4 * hb:4 * hb + 4, f2,
                                            c * P:(c + 1) * P, :].transpose([1, 0, 2])
                            dstp = znbig[:, c * NCOL:(c + 1) * NCOL]
                            dstp = dstp.rearrange("p (i o t) -> p o i t",
                                                  i=4, o=4)[:, off]
                            nc.sync.dma_start(dstp, srcp)
                for c in range(CH):
                    dstp = zbig[:, c * NCOL:(c + 1) * NCOL]
                    dstp = dstp.rearrange("p (t n) -> p t n", n=16)
                    srcp = znbig[:, c * NCOL:(c + 1) * NCOL]
                    srcp = srcp.rearrange("p (n t) -> p n t", t=TLOC)
                    nc.vector.tensor_copy(dstp, srcp.transpose([0, 2, 1]))

            sqpool = p34.enter_context(tc.tile_pool(name="sq3", bufs=4))
            rowpool = p34.enter_context(tc.tile_pool(name="rows3", bufs=7))
            bpool = p34.enter_context(tc.tile_pool(name="bcast3", bufs=2))
            tmppool = p34.enter_context(tc.tile_pool(name="tmp3", bufs=2))
            xhpool = p34.enter_context(tc.tile_pool(name="xh3", bufs=8))
            etpool = p34.enter_context(tc.tile_pool(name="et3", bufs=14))
            brdpool = p34.enter_context(tc.tile_pool(name="brd3", bufs=4))
            z3pool = p34.enter_context(tc.tile_pool(name="z3", bufs=1))
            z3big = z3pool.tile([P, CH * NCOL], BF16D, name="z3big")

            with ExitStack() as ph3:
                w4p = ph3.enter_context(tc.tile_pool(name="w4", bufs=1))
                wq4 = _load_weight(env, w4p, env.d['wq4'], CH, C)
                wk4 = _load_weight(env, w4p, env.d['wk4'], CH, C)
                wv4 = _load_weight(env, w4p, env.d['wv4'], CH, C)
                wp4 = _load_weight(env, w4p, env.d['wp4'], CH, C)
                qk4p = ph3.enter_context(tc.tile_pool(name="qk4", bufs=1))
                q4big = qk4p.tile([P, CH * NCOL], BF16D, name="q4big")
                k4big = qk4p.tile([P, CH * NCOL], BF16D, name="k4big")
                v4p = ph3.enter_context(tc.tile_pool(name="v4", bufs=8))
                av4p = ph3.enter_context(tc.tile_pool(name="av4", bufs=1))
                av4big = av4p.tile([P, CH * NCOL], BF16D, name="av4big")
                vaug4 = [None] * 8
                for cb in range(2):
                    cs = slice(cb * T, (cb + 1) * T)
                    zs = [zbig[:, c * NCOL:(c + 1) * NCOL][:, cs] for c in range(CH)]
                    xh4 = []
                    for c in range(CH):
                        t = xhpool.tile([P, T], BF16D, name="xh4")
                        xh4.append(t)
                    _ln_feat(env, zs, xh4, sqpool, rowpool, bpool, tmppool, in_bf=True)
                    _proj_feat(env, wq4, xh4, 'bq4',
                               lambda co, ps, cb=cb: nc.scalar.activation(
                                   q4big[:, co * NCOL + cb * T:co * NCOL + (cb + 1) * T],
                                   ps, AF.Copy))
                    _proj_feat(env, wk4, xh4, 'bk4',
                               lambda co, ps, cb=cb: nc.scalar.activation(
                                   k4big[:, co * NCOL + cb * T:co * NCOL + (cb + 1) * T],
                                   ps, AF.Copy))
                    vg = _v_aug(env, wv4, xh4, 'bv4', v4p, 4)
                    for g in range(4):
                        vaug4[cb * 4 + g] = vg[g]
                _attn_grouped(env, q4big, k4big, vaug4, av4big, etpool, rowpool,
                              brdpool, ngroups=8)
                for cb in range(2):
                    avs = [av4big[:, c * NCOL + cb * T:c * NCOL + (cb + 1) * T]
                           for c in range(CH)]

                    def evict_z3(co, ps, cb=cb):
                        dst = z3big[:, co * NCOL + cb * T:co * NCOL + (cb + 1) * T]
                        nc.vector.tensor_add(
                            dst, zbig[:, co * NCOL + cb * T:co * NCOL + (cb + 1) * T], ps)

                    _proj_feat(env, wp4, avs, 'bp4', evict_z3)

            # ---- phase 4: MLP ----
            with ExitStack() as ph4:
                wmp = ph4.enter_context(tc.tile_pool(name="wm", bufs=1))
                w1t = _load_weight(env, wmp, env.d['w1'], CH, HID)
                w2t = _load_weight(env, wmp, env.d['w2'], HCH, C)
                h1pool = ph4.enter_context(tc.tile_pool(name="h1", bufs=1))
                opool = ph4.enter_context(tc.tile_pool(name="ostage", bufs=1))
                bh = env.bias.get('bh')
                bo = env.bias.get('bo')
                for cb in range(2):
                    obig = opool.tile([P, CH * T], F32D, name="obig")
                    z3s = [z3big[:, c * NCOL + cb * T:c * NCOL + (cb + 1) * T]
                           for c in range(CH)]
                    xh3 = []
                    for c in range(CH):
                        t = xhpool.tile([P, T], BF16D, name="xh4")
                        xh3.append(t)
                    _ln_feat(env, z3s, xh3, sqpool, rowpool, bpool, tmppool, in_bf=True)
                    h1big = h1pool.tile([P, HCH * T], BF16D, name="h1big")
                    for co in range(HCH):
                        ps = env.psA.tile([P, 512], F32D, name="psA")
                        for ci in range(CH):
                            nc.tensor.matmul(ps, w1t[ci][:, co * P:(co + 1) * P],
                                             xh3[ci], start=(ci == 0),
                                             stop=(ci == CH - 1 and bh is None))
                        if bh is not None:
                            nc.tensor.matmul(ps, bh[0:1, co * P:(co + 1) * P],
                                             env.ones_row[0:1, :], start=False, stop=True)
                        nc.scalar.activation(h1big[:, co * T:(co + 1) * T], ps, AF.Gelu)
                    for co in range(CH):
                        ps = env.psA.tile([P, 512], F32D, name="psA")
                        for ci in range(HCH):
                            nc.tensor.matmul(ps, w2t[ci][:, co * P:(co + 1) * P],
                                             h1big[:, ci * T:(ci + 1) * T],
                                             start=(ci == 0),
                                             stop=(ci == HCH - 1 and bo is None))
                        if bo is not None:
                            nc.tensor.matmul(ps, bo[0:1, co * P:(co + 1) * P],
                                             env.ones_row[0:1, :], start=False, stop=True)
                        nc.vector.tensor_add(
                            obig[:, co * T:(co + 1) * T],
                            z3big[:, co * NCOL + cb * T:co * NCOL + (cb + 1) * T], ps)
                    nc.sync.dma_start(
                        env.d['out'].ap()[:, hb * NCOL + cb * T:hb * NCOL + (cb + 1) * T]
                           .rearrange("(c p) t -> p c t", p=P),
                        obig[:, :].rearrange("p (c t) -> p c t", t=T))


# ----------------------------------------------------------------------------
# program build + host glue
# ----------------------------------------------------------------------------

_CACHE = {}


def build_program(bias_flags_key, no_cc=False, rounds=1):
    cache_key = (bias_flags_key, no_cc, rounds)
    if cache_key in _CACHE:
        return _CACHE[cache_key]
    bias_flags = dict(bias_flags_key)
    nc = bacc.Bacc("TRN2", target_bir_lowering=False, debug=False,
                   num_devices=NCORES)
    env = Env()
    env.nc = nc
    d = {}
    d['xT'] = nc.dram_tensor("xT", [NF, C, T], F32D, kind="ExternalInput")
    d['condT'] = nc.dram_tensor("condT", [C, T], BF16D, kind="ExternalInput")
    for w in WNAMES:
        d[w] = nc.dram_tensor(w, [C, C], BF16D, kind="ExternalInput")
    d['w1'] = nc.dram_tensor("w1", [C, HID], BF16D, kind="ExternalInput")
    d['w2'] = nc.dram_tensor("w2", [HID, C], BF16D, kind="ExternalInput")
    for bn in BNAMES:
        if bias_flags.get(bn):
            d[bn] = nc.dram_tensor(bn, [1, C], F32D, kind="ExternalInput")
    if bias_flags.get('bh'):
        d['bh'] = nc.dram_tensor("bh", [1, HID], F32D, kind="ExternalInput")
    d['mask01'] = nc.dram_tensor("mask01", [P, P], BF16D, kind="ExternalInput")
    d['out'] = nc.dram_tensor("out", [C, 2048], F32D, kind="ExternalOutput")
    for nm in ('in_bufA', 'in_bufB', 'out_bufA', 'out_bufB'):
        d[nm] = nc.dram_tensor(nm, [NCORES, 2, C, TLOC], BF16D)
    d['y1buf'] = nc.dram_tensor("y1buf", [NF, C, T], F32D)
    env.d = d

    with tile.TileContext(nc) as tc:
        for _ in range(rounds):
            with ExitStack() as ctx:
                emit(ctx, tc, env, bias_flags, no_cc=no_cc)
    nc.compile()
    _CACHE[cache_key] = nc
    return nc


NCOL_OUT = 1024


def prep_inputs(x, cond, params):
    """Host-side prep: gamma folding, transposes, per-core sharding."""
    x = np.asarray(x, f32)
    cond = np.asarray(cond, f32)
    g1, be1 = [np.asarray(a, f32) for a in params['ln1']]
    g2, be2 = [np.asarray(a, f32) for a in params['ln2']]
    g3, be3 = [np.asarray(a, f32) for a in params['ln3']]
    g4, be4 = [np.asarray(a, f32) for a in params['ln4']]

    def fold(w, g):
        return np.ascontiguousarray((np.asarray(w, f32) * g[None, :]).T).astype(BF)

    def plain(w):
        return np.ascontiguousarray(np.asarray(w, f32).T).astype(BF)

    def lnbias(w, b):
        return (np.asarray(w, f32) @ b).astype(f32)

    at, cr, a4, ml = params['attn'], params['cross'], params['attn_t'], params['mlp']
    W = {
        'wq1': fold(at['wq'], g1), 'wk1': fold(at['wk'], g1),
        'wv1': fold(at['wv'], g1), 'wp1': plain(at['wp']),
        'wq2': fold(cr['wq'], g2), 'wkc': plain(cr['wk']),
        'wvc': plain(cr['wv']), 'wp2': plain(cr['wp']),
        'wq4': fold(a4['wq'], g4), 'wk4': fold(a4['wk'], g4),
        'wv4': fold(a4['wv'], g4), 'wp4': plain(a4['wp']),
        'w1': fold(ml['w1'], g3), 'w2': plain(ml['w2']),
    }
    Bv = {
        'bq1': lnbias(at['wq'], be1), 'bk1': lnbias(at['wk'], be1),
        'bv1': lnbias(at['wv'], be1), 'bp1': np.asarray(at['bp'], f32),
        'bq2': lnbias(cr['wq'], be2), 'bp2': np.asarray(cr['bp'], f32),
        'bq4': lnbias(a4['wq'], be4), 'bk4': lnbias(a4['wk'], be4),
        'bv4': lnbias(a4['wv'], be4), 'bp4': np.asarray(a4['bp'], f32),
        'bh': (lnbias(ml['w1'], be3) + np.asarray(ml['b1'], f32)),
        'bo': np.asarray(ml['b2'], f32),
    }
    bias_flags = {k: bool(np.any(v != 0)) for k, v in Bv.items()}

    mask01 = np.zeros((P, P), BF)
    for blk in range(8):
        mask01[blk * 16:(blk + 1) * 16, blk * 16:(blk + 1) * 16] = 1

    in_maps = []
    for i in range(NCORES):
        b, s = i // 4, i % 4
        m = {
            'xT': np.ascontiguousarray(x[b, 4 * s:4 * s + 4].transpose(0, 2, 1)),
            'condT': np.ascontiguousarray(cond[b].T).astype(BF),
            'mask01': mask01,
        }
        m.update(W)
        for k, fl in bias_flags.items():
            if fl:
                m[k] = np.ascontiguousarray(Bv[k][None, :])
        in_maps.append(m)
    return in_maps, bias_flags


def assemble_output(results):
    """results: list of per-core dicts with 'out' [768, 2048] f32."""
    B, N, Tfull = 2, 16, 512
    out = np.zeros((B, N, Tfull, C), f32)
    for j in range(NCORES):
        o = results[j]['out']   # [C, 2048]
        oz = o.reshape(C, 2, TLOC, 16).transpose(1, 3, 2, 0)
        out[:, :, TLOC * j:TLOC * (j + 1), :] = oz
    return out


LAST_RESULTS = None


def kernel(x, cond, params):
    global LAST_RESULTS
    in_maps, bias_flags = prep_inputs(x, cond, params)
    nc = build_program(tuple(sorted(bias_flags.items())))
    res = run_bass_kernel_spmd(nc, in_maps, list(range(NCORES)))
    LAST_RESULTS = res
    return assemble_output(res.results)


if __name__ == "__main__":
    import reference
    inputs = reference.setup_inputs()
    out = kernel(**{k: v for k, v in inputs.items()})
    print("kernel ran; out shape", out.shape)


# revision 29
# speedup vs baseline: 1.4521x; 1.3580x over previous
"""Trainium2 Bass kernel for nn_Block_49220325212407 (dense transformer block).

Strategy (8 NeuronCores, SPMD single NEFF):
- Phases 1/2 (self-attn over T, cross-attn): data-parallel over the 32
  (b, n) frames -> 4 frames per core.
- One 8-way AllToAll (split in two halves for overlap) reshards to
  (b, t-chunk) ownership: core j owns t-rows [64j, 64j+64) of all frames.
- Phase 3 (frame attention over N=16): block-diagonal batched attention,
  8 t-rows per 128-col group, multiplicative 0/1 mask after exp.
- Phase 4 (MLP): per-token, computed in the resharded layout.

All activations live feature-major ([C, tokens] on SBUF partitions), so no
transposes are ever needed: LN stats via ones-column matmuls, broadcasts via
GpSimd partition_broadcast, softmax denominators via a ones-column appended
to V (v_aug), and the av matmul produces feature-major output directly.
Matmuls run in bf16 with f32 PSUM accumulation; the residual stream stays
f32 through phase 1 and is rounded to bf16 at the AllToAll and after
phase 3 (rel err ~6e-3 vs the f32 reference).
"""
import os
from contextlib import ExitStack

import numpy as np
import ml_dtypes

import concourse.bass as bass
import concourse.bass_isa as bass_isa
import concourse.tile as tile
from concourse import bacc, mybir
from concourse.bass_utils import run_bass_kernel_spmd

f32 = np.float32
BF = ml_dtypes.bfloat16
F32D = mybir.dt.float32
BF16D = mybir.dt.bfloat16
AF = mybir.ActivationFunctionType

P = 128
C = 768
CH = 6          # C / P
T = 512
H = 12
D = 64
NF = 4          # frames per core
NCORES = 8
TLOC = T // NCORES   # 64
HID = 3072
HCH = 24        # HID / P
EPS = 1e-5
SCALE = 0.125   # d ** -0.5

WNAMES = ['wq1', 'wk1', 'wv1', 'wp1', 'wq2', 'wkc', 'wvc', 'wp2',
          'wq4', 'wk4', 'wv4', 'wp4']
BNAMES = ['bq1', 'bk1', 'bv1', 'bp1', 'bq2', 'bp2',
          'bq4', 'bk4', 'bv4', 'bp4', 'bo']


# ----------------------------------------------------------------------------
# emission helpers
# ----------------------------------------------------------------------------

class Env:
    """Holds nc/tc, dram handles, const tiles, psum pools."""
    pass


def _ln_feat(env, x_slices, xh_out, sqpool, rowpool, bpool, tmppool, in_bf):
    """Feature-major LayerNorm (gamma/beta folded into downstream weights).

    Cross-partition sums run on GpSimd (partition_all_reduce broadcasts the
    sum to every partition), keeping the PE free.
    x_slices: 6 APs [128, Tq] (f32 or bf16); xh_out: 6 APs [128, Tq] bf16.
    """
    nc = env.nc
    Tq = x_slices[0].shape[1]
    xsum = sqpool.tile([P, 512], F32D, name="sq")[:, :Tq]
    nc.vector.tensor_add(xsum, x_slices[0], x_slices[1])
    for c in range(2, CH):
        nc.vector.tensor_add(xsum, xsum, x_slices[c])
    sqsum = sqpool.tile([P, 512], F32D, name="sq")[:, :Tq]
    nc.vector.tensor_mul(sqsum, x_slices[0], x_slices[0])
    for c in range(1, CH):
        sq = sqpool.tile([P, 512], F32D, name="sq")[:, :Tq]
        nc.vector.tensor_mul(sq, x_slices[c], x_slices[c])
        nc.vector.tensor_add(sqsum, sqsum, sq)
    bm = bpool.tile([P, 512], F32D, name="s1b")[:, :Tq]
    s2b = bpool.tile([P, 512], F32D, name="s2b")[:, :Tq]
    nc.gpsimd.partition_all_reduce(bm, xsum, channels=P,
                                   reduce_op=bass_isa.ReduceOp.add)
    nc.gpsimd.partition_all_reduce(s2b, sqsum, channels=P,
                                   reduce_op=bass_isa.ReduceOp.add)
    nc.vector.tensor_scalar_mul(bm, bm, 1.0 / C)   # bm = mean, all partitions
    m2row = rowpool.tile([1, 512], F32D, name="row")[:, :Tq]
    vrow = rowpool.tile([1, 512], F32D, name="row")[:, :Tq]
    srow = rowpool.tile([1, 512], F32D, name="row")[:, :Tq]
    rrow = rowpool.tile([1, 512], F32D, name="row")[:, :Tq]
    nc.vector.tensor_mul(m2row, bm[0:1, :], bm[0:1, :])
    nc.vector.tensor_scalar_mul(vrow, s2b[0:1, :], 1.0 / C)
    nc.vector.tensor_sub(vrow, vrow, m2row)
    nc.scalar.activation(srow, vrow, AF.Sqrt, bias=env.eps_tile[0:1, 0:1])
    nc.vector.reciprocal(rrow, srow)
    brs = bpool.tile([P, 512], F32D, name="brs")[:, :Tq]
    nc.gpsimd.partition_broadcast(brs, rrow)
    for c in range(CH):
        tmp = tmppool.tile([P, 512], F32D, name="lntmp")[:, :Tq]
        nc.vector.tensor_sub(tmp, x_slices[c], bm)
        nc.vector.tensor_mul(xh_out[c], tmp, brs)


def _proj_feat(env, wtiles, rhs_slices, bias_name, evict, nco=CH, Tq=T):
    """out^T[co-chunk] = sum_ci W[ci][:, co].T @ rhs[ci]  (+ bias row).

    evict(co, ps): consume psum tile [128, Tq]."""
    nc = env.nc
    bias = env.bias.get(bias_name)
    for co in range(nco):
        ps = env.psA.tile([P, 512], F32D, name="psA")[:, :Tq]
        for ci in range(len(rhs_slices)):
            nc.tensor.matmul(ps, wtiles[ci][:, co * P:(co + 1) * P], rhs_slices[ci],
                             start=(ci == 0),
                             stop=(ci == len(rhs_slices) - 1 and bias is None))
        if bias is not None:
            nc.tensor.matmul(ps, bias[0:1, co * P:(co + 1) * P],
                             env.ones_row[0:1, :Tq], start=False, stop=True)
        evict(co, ps)


def _v_aug(env, wtiles, xh_slices, bias_name, vpool, jcs, Tq=T, name="vaug"):
    """Token-major V with ones column per head: jcs tiles [128, 780] bf16."""
    nc = env.nc
    bias = env.bias.get(bias_name)
    vaug = []
    for jc in range(jcs):
        vt = vpool.tile([P, H * 65], BF16D, name=name if name == "vaug" else f"{name}{jc}")
        nc.vector.memset(vt[:], 1.0)
        vaug.append(vt)
    for jc in range(jcs):
        for cog in range(2):
            ps = env.psA.tile([P, 512], F32D, name="psA")[:, :384]
            for ci in range(CH):
                nc.tensor.matmul(
                    ps, xh_slices[ci][:, jc * P:(jc + 1) * P],
                    wtiles[ci][:, cog * 384:(cog + 1) * 384],
                    start=(ci == 0), stop=(ci == CH - 1 and bias is None))
            if bias is not None:
                nc.tensor.matmul(ps, env.ones_row[0:1, 0:P],
                                 bias[0:1, cog * 384:(cog + 1) * 384],
                                 start=False, stop=True)
            dst = vaug[jc][:, :].rearrange("p (h x) -> p h x", x=65)
            dst = dst[:, cog * 6:(cog + 1) * 6, 0:64]
            src = ps.rearrange("p (h x) -> p h x", x=64)
            nc.scalar.activation(dst, src, AF.Copy)
    return vaug


def _attn_full(env, qbig, kbig, vaug, avbig, etpool, rowpool, brdpool):
    """Phase-1/2 attention: full-T (512 keys, 4 j-chunks).

    Heads are processed in even/odd pairs: the K=64 sim matmuls of the pair
    run as concurrent 64x128 row-tiles (partitions 0-63 / 64-127), and all
    sims are issued before all avs so the PE tiling mode only switches twice
    per pair instead of per matmul."""
    nc = env.nc
    for hq in range(H // 4):   # 4 heads per batch: 2 pairs
        ets = {}
        for hpi in (0, 1):
            hp = 2 * hq + hpi
            co = hp
            ets[hpi] = {0: [], 64: []}
            for jc in range(4):
                for po in (0, 64):
                    qh = qbig[po:po + 64, co * T:(co + 1) * T]
                    kh = kbig[po:po + 64, co * T:(co + 1) * T]
                    ps = env.psS.tile([P, 512], F32D, name="psS")
                    nc.tensor.matmul(ps, kh[:, jc * P:(jc + 1) * P], qh,
                                     start=True, stop=True)
                    et = etpool.tile([P, 512], BF16D, name="et")
                    nc.scalar.activation(et, ps, AF.Exp, scale=SCALE)
                    ets[hpi][po].append(et)
        for hpi in (0, 1):
            hp = 2 * hq + hpi
            co = hp
            for po in (0, 64):
                h = 2 * hp + (1 if po else 0)
                av = env.psS.tile([P, 512], F32D, name="psS")
                for jc in range(4):
                    nc.tensor.matmul(av[0:65, :], vaug[jc][:, 65 * h:65 * h + 65],
                                     ets[hpi][po][jc], start=(jc == 0), stop=(jc == 3))
                rr = rowpool.tile([1, 512], F32D, name="row")
                nc.vector.reciprocal(rr, av[64:65, :])
                brd = brdpool.tile([P, 512], F32D, name="brd")
                nc.gpsimd.partition_broadcast(brd, rr)
                dst = avbig[po:po + 64, co * T:(co + 1) * T]
                nc.scalar.activation(dst, av[0:64, :], AF.Copy)
                nc.vector.tensor_mul(dst, dst, brd[po:po + 64, :])


def _attn_grouped(env, qbig, kbig, vaug, avbig, etpool, rowpool, brdpool, ngroups):
    """Phase-3 frame attention: 128-col groups, block-diag mask applied to exp.

    Per group: all 12 heads' sims first (paired 64x128 row-tiles), then all
    avs, so the PE tiling mode switches twice per group."""
    nc = env.nc
    ncols = ngroups * P
    for gp in range(ngroups // 2):
        ets = {}
        for g in (2 * gp, 2 * gp + 1):
            for h in range(H):
                po = 64 * (h % 2)
                co = h // 2
                qh = qbig[po:po + 64, co * ncols + g * P: co * ncols + (g + 1) * P]
                kh = kbig[po:po + 64, co * ncols + g * P: co * ncols + (g + 1) * P]
                ps = env.psS.tile([P, 512], F32D, name="psS")[:, :P]
                nc.tensor.matmul(ps, kh, qh, start=True, stop=True)
                et = etpool.tile([P, P], BF16D, name="et")
                nc.scalar.activation(et, ps, AF.Exp, scale=SCALE)
                nc.vector.tensor_mul(et, et, env.mask)
                ets[(g, h)] = et
        for g in (2 * gp, 2 * gp + 1):
            for h in range(H):
                po = 64 * (h % 2)
                co = h // 2
                av = env.psS.tile([P, 512], F32D, name="psS")[:, :P]
                nc.tensor.matmul(av[0:65, :], vaug[g][:, 65 * h:65 * h + 65],
                                 ets[(g, h)], start=True, stop=True)
                rr = rowpool.tile([1, 512], F32D, name="row")[:, :P]
                nc.vector.reciprocal(rr, av[64:65, :])
                brd = brdpool.tile([P, 512], F32D, name="brd")[:, :P]
                nc.gpsimd.partition_broadcast(brd, rr)
                dst = avbig[po:po + 64, co * ncols + g * P: co * ncols + (g + 1) * P]
                nc.scalar.activation(dst, av[0:64, :], AF.Copy)
                nc.vector.tensor_mul(dst, dst, brd[po:po + 64, :])


def _load_weight(env, pool, dram, nci, width):
    """One DMA for the whole weight: [nci*128, width] -> [128, nci*width]."""
    big = pool.tile([P, nci * width], BF16D, name=f"{dram.name}_w")
    dst = big[:, :].rearrange("p (c w) -> p c w", w=width)
    srcr = dram.ap().rearrange("(c p) w -> p c w", p=P)
    env.nc.sync.dma_start(dst, srcr)
    return [big[:, ci * width:(ci + 1) * width] for ci in range(nci)]


# ----------------------------------------------------------------------------
# main emission
# ----------------------------------------------------------------------------

def emit(ctx, tc, env, bias_flags, no_cc=False):
    nc = env.nc

    def _a2a(in_d, out_d):
        if no_cc:
            nc.sync.dma_start(out_d.ap()[:, :, :, :], in_d.ap()[:, :, :, :])
        else:
            nc.gpsimd.collective_compute(
                "AllToAll", mybir.AluOpType.bypass,
                replica_groups=[list(range(NCORES))],
                ins=[in_d.ap()[:, :, :, :]],
                outs=[out_d.ap()[:, :, :, :]])

    constp = ctx.enter_context(tc.tile_pool(name="const", bufs=1))
    env.ones_col_f32 = constp.tile([P, 1], F32D, name="ones_col_f32")
    nc.vector.memset(env.ones_col_f32[:], 1.0)
    env.ones_col_bf = constp.tile([P, 1], BF16D, name="ones_col_bf")
    nc.vector.memset(env.ones_col_bf[:], 1.0)
    env.ones_row = constp.tile([1, 512], F32D, name="ones_row")
    nc.vector.memset(env.ones_row[:], 1.0)
    env.eps_tile = constp.tile([1, 1], F32D, name="eps_tile")
    nc.vector.memset(env.eps_tile[:], EPS)
    env.mask = constp.tile([P, P], BF16D, name="maskt")
    nc.sync.dma_start(env.mask[:], env.d['mask01'].ap()[:, :])
    env.bias = {}
    for bn in BNAMES + ['bh']:
        if bias_flags.get(bn):
            width = HID if bn == 'bh' else C
            bt = constp.tile([1, width], F32D, name=f"{bn}_t")
            nc.sync.dma_start(bt[:], env.d[bn].ap()[:, :])
            env.bias[bn] = bt

    # PSUM budget: psA 1tag*3 + psS 1tag*4 = 7 banks
    env.psA = ctx.enter_context(tc.tile_pool(name="psA", bufs=3, space="PSUM"))
    env.psS = ctx.enter_context(tc.tile_pool(name="psS", bufs=5, space="PSUM"))

    # ---------------- phases 1 + 2 ----------------
    with ExitStack() as p12:
        sqpool = p12.enter_context(tc.tile_pool(name="sq", bufs=4))
        rowpool = p12.enter_context(tc.tile_pool(name="rows", bufs=7))
        bpool = p12.enter_context(tc.tile_pool(name="bcast", bufs=2))
        tmppool = p12.enter_context(tc.tile_pool(name="tmp", bufs=2))
        xhpool = p12.enter_context(tc.tile_pool(name="xh", bufs=9))
        qpool = p12.enter_context(tc.tile_pool(name="q", bufs=2))
        vpool = p12.enter_context(tc.tile_pool(name="vaug", bufs=6))
        etpool = p12.enter_context(tc.tile_pool(name="et", bufs=16))
        brdpool = p12.enter_context(tc.tile_pool(name="brd", bufs=4))
        avpool = p12.enter_context(tc.tile_pool(name="av", bufs=2))
        ypool = p12.enter_context(tc.tile_pool(name="ystage", bufs=4))
        # per-frame residual stream tiles; spilled to DRAM between the phases
        xfpool = p12.enter_context(tc.tile_pool(name="xf", bufs=2))

        # ---- phase 1: self-attention over T, per frame ----
        with ExitStack() as ph1:
            wself = ph1.enter_context(tc.tile_pool(name="wself", bufs=1))
            kpool = ph1.enter_context(tc.tile_pool(name="k", bufs=2))
            wq1 = _load_weight(env, wself, env.d['wq1'], CH, C)
            wk1 = _load_weight(env, wself, env.d['wk1'], CH, C)
            wv1 = _load_weight(env, wself, env.d['wv1'], CH, C)
            wp1 = _load_weight(env, wself, env.d['wp1'], CH, C)
            for f in range(NF):
                xfbig = xfpool.tile([P, CH * T], F32D, name="xf")
                nc.sync.dma_start(
                    xfbig[:, :].rearrange("p (c t) -> p c t", t=T),
                    env.d['xT'].ap()[f].rearrange("(c p) t -> p c t", p=P))
                xs = [xfbig[:, c * T:(c + 1) * T] for c in range(CH)]
                xh = []
                for c in range(CH):
                    t = xhpool.tile([P, T], BF16D, name="xh")
                    xh.append(t)
                _ln_feat(env, xs, xh, sqpool, rowpool, bpool, tmppool, in_bf=False)
                qbig = qpool.tile([P, CH * T], BF16D, name="qbig")
                kbig = kpool.tile([P, CH * T], BF16D, name="kbig")
                _proj_feat(env, wq1, xh, 'bq1',
                           lambda co, ps: nc.scalar.activation(
                               qbig[:, co * T:(co + 1) * T], ps, AF.Copy))
                _proj_feat(env, wk1, xh, 'bk1',
                           lambda co, ps: nc.scalar.activation(
                               kbig[:, co * T:(co + 1) * T], ps, AF.Copy))
                vaug = _v_aug(env, wv1, xh, 'bv1', vpool, 4)
                avbig = avpool.tile([P, CH * T], BF16D, name="avbig")
                _attn_full(env, qbig, kbig, vaug, avbig, etpool, rowpool, brdpool)
                avs = [avbig[:, c * T:(c + 1) * T] for c in range(CH)]

                def evict_y1(co, ps, xs=xs):
                    nc.vector.tensor_add(xs[co], xs[co], ps)

                _proj_feat(env, wp1, avs, 'bp1', evict_y1)
                nc.sync.dma_start(
                    env.d['y1buf'].ap()[f].rearrange("(c p) t -> p c t", p=P),
                    xfbig[:, :].rearrange("p (c t) -> p c t", t=T))

        # ---- phase 2: cross-attention, per frame ----
        with ExitStack() as ph2:
            wcross = ph2.enter_context(tc.tile_pool(name="wcross", bufs=1))
            wq2 = _load_weight(env, wcross, env.d['wq2'], CH, C)
            wkc = _load_weight(env, wcross, env.d['wkc'], CH, C)
            wvc = _load_weight(env, wcross, env.d['wvc'], CH, C)
            wp2 = _load_weight(env, wcross, env.d['wp2'], CH, C)
            condbig = wcross.tile([P, CH * T], BF16D, name="condbig")
            nc.sync.dma_start(
                condbig[:, :].rearrange("p (c t) -> p c t", t=T),
                env.d['condT'].ap().rearrange("(c p) t -> p c t", p=P))
            condb = [condbig[:, c * T:(c + 1) * T] for c in range(CH)]
            kcbig = wcross.tile([P, CH * T], BF16D, name="kcbig")
            _proj_feat(env, wkc, condb, None,
                       lambda co, ps: nc.scalar.activation(
                           kcbig[:, co * T:(co + 1) * T], ps, AF.Copy))
            vcaug = _v_aug(env, wvc, condb, None, wcross, 4, name="vc")

            for f in range(NF):
                xfbig = xfpool.tile([P, CH * T], F32D, name="xf")
                nc.sync.dma_start(
                    xfbig[:, :].rearrange("p (c t) -> p c t", t=T),
                    env.d['y1buf'].ap()[f].rearrange("(c p) t -> p c t", p=P))
                xs = [xfbig[:, c * T:(c + 1) * T] for c in range(CH)]
                xh = []
                for c in range(CH):
                    t = xhpool.tile([P, T], BF16D, name="xh")
                    xh.append(t)
                _ln_feat(env, xs, xh, sqpool, rowpool, bpool, tmppool, in_bf=False)
                qbig = qpool.tile([P, CH * T], BF16D, name="qbig")
                _proj_feat(env, wq2, xh, 'bq2',
                           lambda co, ps: nc.scalar.activation(
                               qbig[:, co * T:(co + 1) * T], ps, AF.Copy))
                avbig = avpool.tile([P, CH * T], BF16D, name="avbig")
                _attn_full(env, qbig, kcbig, vcaug, avbig, etpool, rowpool, brdpool)
                avs = [avbig[:, c * T:(c + 1) * T] for c in range(CH)]

                buf = env.d['in_bufA'] if f < 2 else env.d['in_bufB']
                fb = f % 2

                def evict_y2(co, ps, buf=buf, fb=fb, xs=xs):
                    yst = ypool.tile([P, T], BF16D, name="ystage")
                    nc.vector.tensor_add(yst, xs[co], ps)
                    dst = buf.ap()[:, fb, co * P:(co + 1) * P, :].transpose([1, 0, 2])
                    src = yst[:, :].rearrange("c (j t) -> c j t", j=NCORES)
                    nc.sync.dma_start(dst, src)

                _proj_feat(env, wp2, avs, 'bp2', evict_y2)

                if f == 1:
                    _a2a(env.d['in_bufA'], env.d['out_bufA'])
            _a2a(env.d['in_bufB'], env.d['out_bufB'])

    # ---------------- phases 3 + 4, per b-half ----------------
    NCOL = 1024   # columns per half: col = t*16 + n
    for hb in range(2):
        with ExitStack() as p34:
            zpool = p34.enter_context(tc.tile_pool(name="zpool", bufs=1))
            zbig = zpool.tile([P, CH * NCOL], BF16D, name="zbig")   # freed use after ph3
            # load frame-major (contiguous DMA), then repack to t-major on DVE:
            # zbig[p, t*16+n] = znbig[p, n*64+t]
            with ExitStack() as zl:
                znpool = zl.enter_context(tc.tile_pool(name="zn", bufs=1))
                znbig = znpool.tile([P, CH * NCOL], BF16D, name="znbig")
                for fb2, buf in ((0, env.d['out_bufA']), (1, env.d['out_bufB'])):
                    for f2 in range(2):
                        off = 2 * fb2 + f2     # n = 4*i4 + off
                        for c in range(CH):
                            srcp = buf.ap()[4 * hb:4 * hb + 4, f2,
                                            c * P:(c + 1) * P, :].transpose([1, 0, 2])
                            dstp = znbig[:, c * NCOL:(c + 1) * NCOL]
                            dstp = dstp.rearrange("p (i o t) -> p o i t",
                                                  i=4, o=4)[:, off]
                            nc.sync.dma_start(dstp, srcp)
                for c in range(CH):
                    dstp = zbig[:, c * NCOL:(c + 1) * NCOL]
                    dstp = dstp.rearrange("p (t n) -> p t n", n=16)
                    srcp = znbig[:, c * NCOL:(c + 1) * NCOL]
                    srcp = srcp.rearrange("p (n t) -> p n t", t=TLOC)
                    nc.vector.tensor_copy(dstp, srcp.transpose([0, 2, 1]))

            sqpool = p34.enter_context(tc.tile_pool(name="sq3", bufs=4))
            rowpool = p34.enter_context(tc.tile_pool(name="rows3", bufs=7))
            bpool = p34.enter_context(tc.tile_pool(name="bcast3", bufs=2))
            tmppool = p34.enter_context(tc.tile_pool(name="tmp3", bufs=2))
            xhpool = p34.enter_context(tc.tile_pool(name="xh3", bufs=8))
            etpool = p34.enter_context(tc.tile_pool(name="et3", bufs=26))
            brdpool = p34.enter_context(tc.tile_pool(name="brd3", bufs=4))
            z3pool = p34.enter_context(tc.tile_pool(name="z3", bufs=1))
            z3big = z3pool.tile([P, CH * NCOL], BF16D, name="z3big")

            with ExitStack() as ph3:
                w4p = ph3.enter_context(tc.tile_pool(name="w4", bufs=1))
                wq4 = _load_weight(env, w4p, env.d['wq4'], CH, C)
                wk4 = _load_weight(env, w4p, env.d['wk4'], CH, C)
                wv4 = _load_weight(env, w4p, env.d['wv4'], CH, C)
                wp4 = _load_weight(env, w4p, env.d['wp4'], CH, C)
                qk4p = ph3.enter_context(tc.tile_pool(name="qk4", bufs=1))
                q4big = qk4p.tile([P, CH * NCOL], BF16D, name="q4big")
                k4big = qk4p.tile([P, CH * NCOL], BF16D, name="k4big")
                v4p = ph3.enter_context(tc.tile_pool(name="v4", bufs=8))
                av4p = ph3.enter_context(tc.tile_pool(name="av4", bufs=1))
                av4big = av4p.tile([P, CH * NCOL], BF16D, name="av4big")
                vaug4 = [None] * 8
                for cb in range(2):
                    cs = slice(cb * T, (cb + 1) * T)
                    zs = [zbig[:, c * NCOL:(c + 1) * NCOL][:, cs] for c in range(CH)]
                    xh4 = []
                    for c in range(CH):
                        t = xhpool.tile([P, T], BF16D, name="xh4")
                        xh4.append(t)
                    _ln_feat(env, zs, xh4, sqpool, rowpool, bpool, tmppool, in_bf=True)
                    _proj_feat(env, wq4, xh4, 'bq4',
                               lambda co, ps, cb=cb: nc.scalar.activation(
                                   q4big[:, co * NCOL + cb * T:co * NCOL + (cb + 1) * T],
                                   ps, AF.Copy))
                    _proj_feat(env, wk4, xh4, 'bk4',
                               lambda co, ps, cb=cb: nc.scalar.activation(
                                   k4big[:, co * NCOL + cb * T:co * NCOL + (cb + 1) * T],
                                   ps, AF.Copy))
                    vg = _v_aug(env, wv4, xh4, 'bv4', v4p, 4)
                    for g in range(4):
                        vaug4[cb * 4 + g] = vg[g]
                _attn_grouped(env, q4big, k4big, vaug4, av4big, etpool, rowpool,
                              brdpool, ngroups=8)
                for cb in range(2):
                    avs = [av4big[:, c * NCOL + cb * T:c * NCOL + (cb + 1) * T]
                           for c in range(CH)]

                    def evict_z3(co, ps, cb=cb):
                        dst = z3big[:, co * NCOL + cb * T:co * NCOL + (cb + 1) * T]
                        nc.vector.tensor_add(
                            dst, zbig[:, co * NCOL + cb * T:co * NCOL + (cb + 1) * T], ps)

                    _proj_feat(env, wp4, avs, 'bp4', evict_z3)

            # ---- phase 4: MLP ----
            with ExitStack() as ph4:
                wmp = ph4.enter_context(tc.tile_pool(name="wm", bufs=1))
                w1t = _load_weight(env, wmp, env.d['w1'], CH, HID)
                w2t = _load_weight(env, wmp, env.d['w2'], HCH, C)
                h1pool = ph4.enter_context(tc.tile_pool(name="h1", bufs=1))
                opool = ph4.enter_context(tc.tile_pool(name="ostage", bufs=1))
                bh = env.bias.get('bh')
                bo = env.bias.get('bo')
                for cb in range(2):
                    obig = opool.tile([P, CH * T], F32D, name="obig")
                    z3s = [z3big[:, c * NCOL + cb * T:c * NCOL + (cb + 1) * T]
                           for c in range(CH)]
                    xh3 = []
                    for c in range(CH):
                        t = xhpool.tile([P, T], BF16D, name="xh4")
                        xh3.append(t)
                    _ln_feat(env, z3s, xh3, sqpool, rowpool, bpool, tmppool, in_bf=True)
                    h1big = h1pool.tile([P, HCH * T], BF16D, name="h1big")
                    for co in range(HCH):
                        ps = env.psA.tile([P, 512], F32D, name="psA")
                        for ci in range(CH):
                            nc.tensor.matmul(ps, w1t[ci][:, co * P:(co + 1) * P],
                                             xh3[ci], start=(ci == 0),
                                             stop=(ci == CH - 1 and bh is None))
                        if bh is not None:
                            nc.tensor.matmul(ps, bh[0:1, co * P:(co + 1) * P],
                                             env.ones_row[0:1, :], start=False, stop=True)
                        nc.scalar.activation(h1big[:, co * T:(co + 1) * T], ps, AF.Gelu)
                    for co in range(CH):
                        ps = env.psA.tile([P, 512], F32D, name="psA")
                        for ci in range(HCH):
                            nc.tensor.matmul(ps, w2t[ci][:, co * P:(co + 1) * P],
                                             h1big[:, ci * T:(ci + 1) * T],
                                             start=(ci == 0),
                                             stop=(ci == HCH - 1 and bo is None))
                        if bo is not None:
                            nc.tensor.matmul(ps, bo[0:1, co * P:(co + 1) * P],
                                             env.ones_row[0:1, :], start=False, stop=True)
                        nc.vector.tensor_add(
                            obig[:, co * T:(co + 1) * T],
                            z3big[:, co * NCOL + cb * T:co * NCOL + (cb + 1) * T], ps)
                    nc.sync.dma_start(
                        env.d['out'].ap()[:, hb * NCOL + cb * T:hb * NCOL + (cb + 1) * T]
                           .rearrange("(c p) t -> p c t", p=P),
                        obig[:, :].rearrange("p (c t) -> p c t", t=T))


# ----------------------------------------------------------------------------
# program build + host glue
# ----------------------------------------------------------------------------

_CACHE = {}


def build_program(bias_flags_key, no_cc=False, rounds=1):
    cache_key = (bias_flags_key, no_cc, rounds)
    if cache_key in _CACHE:
        return _CACHE[cache_key]
    bias_flags = dict(bias_flags_key)
    nc = bacc.Bacc("TRN2", target_bir_lowering=False, debug=False,
                   num_devices=NCORES)
    env = Env()
    env.nc = nc
    d = {}
    d['xT'] = nc.dram_tensor("xT", [NF, C, T], F32D, kind="ExternalInput")
    d['condT'] = nc.dram_tensor("condT", [C, T], BF16D, kind="ExternalInput")
    for w in WNAMES:
        d[w] = nc.dram_tensor(w, [C, C], BF16D, kind="ExternalInput")
    d['w1'] = nc.dram_tensor("w1", [C, HID], BF16D, kind="ExternalInput")
    d['w2'] = nc.dram_tensor("w2", [HID, C], BF16D, kind="ExternalInput")
    for bn in BNAMES:
        if bias_flags.get(bn):
            d[bn] = nc.dram_tensor(bn, [1, C], F32D, kind="ExternalInput")
    if bias_flags.get('bh'):
        d['bh'] = nc.dram_tensor("bh", [1, HID], F32D, kind="ExternalInput")
    d['mask01'] = nc.dram_tensor("mask01", [P, P], BF16D, kind="ExternalInput")
    d['out'] = nc.dram_tensor("out", [C, 2048], F32D, kind="ExternalOutput")
    for nm in ('in_bufA', 'in_bufB', 'out_bufA', 'out_bufB'):
        d[nm] = nc.dram_tensor(nm, [NCORES, 2, C, TLOC], BF16D)
    d['y1buf'] = nc.dram_tensor("y1buf", [NF, C, T], F32D)
    env.d = d

    with tile.TileContext(nc) as tc:
        for _ in range(rounds):
            with ExitStack() as ctx:
                emit(ctx, tc, env, bias_flags, no_cc=no_cc)
    nc.compile()
    _CACHE[cache_key] = nc
    return nc


NCOL_OUT = 1024


def prep_inputs(x, cond, params):
    """Host-side prep: gamma folding, transposes, per-core sharding."""
    x = np.asarray(x, f32)
    cond = np.asarray(cond, f32)
    g1, be1 = [np.asarray(a, f32) for a in params['ln1']]
    g2, be2 = [np.asarray(a, f32) for a in params['ln2']]
    g3, be3 = [np.asarray(a, f32) for a in params['ln3']]
    g4, be4 = [np.asarray(a, f32) for a in params['ln4']]

    def fold(w, g):
        return np.ascontiguousarray((np.asarray(w, f32) * g[None, :]).T).astype(BF)

    def plain(w):
        return np.ascontiguousarray(np.asarray(w, f32).T).astype(BF)

    def lnbias(w, b):
        return (np.asarray(w, f32) @ b).astype(f32)

    at, cr, a4, ml = params['attn'], params['cross'], params['attn_t'], params['mlp']
    W = {
        'wq1': fold(at['wq'], g1), 'wk1': fold(at['wk'], g1),
        'wv1': fold(at['wv'], g1), 'wp1': plain(at['wp']),
        'wq2': fold(cr['wq'], g2), 'wkc': plain(cr['wk']),
        'wvc': plain(cr['wv']), 'wp2': plain(cr['wp']),
        'wq4': fold(a4['wq'], g4), 'wk4': fold(a4['wk'], g4),
        'wv4': fold(a4['wv'], g4), 'wp4': plain(a4['wp']),
        'w1': fold(ml['w1'], g3), 'w2': plain(ml['w2']),
    }
    Bv = {
        'bq1': lnbias(at['wq'], be1), 'bk1': lnbias(at['wk'], be1),
        'bv1': lnbias(at['wv'], be1), 'bp1': np.asarray(at['bp'], f32),
        'bq2': lnbias(cr['wq'], be2), 'bp2': np.asarray(cr['bp'], f32),
        'bq4': lnbias(a4['wq'], be4), 'bk4': lnbias(a4['wk'], be4),
        'bv4': lnbias(a4['wv'], be4), 'bp4': np.asarray(a4['bp'], f32),
        'bh': (lnbias(ml['w1'], be3) + np.asarray(ml['b1'], f32)),
        'bo': np.asarray(ml['b2'], f32),
    }
    bias_flags = {k: bool(np.any(v != 0)) for k, v in Bv.items()}

    mask01 = np.zeros((P, P), BF)
    for blk in range(8):
        mask01[blk * 16:(blk + 1) * 16, blk * 16:(blk + 1) * 16] = 1

    in_maps = []
    for i in range(NCORES):
        b, s = i // 4, i % 4
        m = {
            'xT': np.ascontiguousarray(x[b, 4 * s:4 * s + 4].transpose(0, 2, 1)),
            'condT': np.ascontiguousarray(cond[b].T).astype(BF),
            'mask01': mask01,
        }
        m.update(W)
        for k, fl in bias_flags.items():
            if fl:
                m[k] = np.ascontiguousarray(Bv[k][None, :])
        in_maps.append(m)
    return in_maps, bias_flags


def assemble_output(results):
    """results: list of per-core dicts with 'out' [768, 2048] f32."""
    B, N, Tfull = 2, 16, 512
    out = np.zeros((B, N, Tfull, C), f32)
    for j in range(NCORES):
        o = results[j]['out']   # [C, 2048]
        oz = o.reshape(C, 2, TLOC, 16).transpose(1, 3, 2, 0)
        out[:, :, TLOC * j:TLOC * (j + 1), :] = oz
    return out


LAST_RESULTS = None


def kernel(x, cond, params):
    global LAST_RESULTS
    in_maps, bias_flags = prep_inputs(x, cond, params)
    nc = build_program(tuple(sorted(bias_flags.items())))
    res = run_bass_kernel_spmd(nc, in_maps, list(range(NCORES)))
    LAST_RESULTS = res
    return assemble_output(res.results)


if __name__ == "__main__":
    import reference
    inputs = reference.setup_inputs()
    out = kernel(**{k: v for k, v in inputs.items()})
    print("kernel ran; out shape", out.shape)
